# revision 1
# baseline (speedup 1.0000x reference)
"""MultiHeadAttention (B=4, S=2048, E=1024, H=16) on 8 Trainium2 NeuronCores.

Sharding: core = batch*2 + head_group; each core computes 8 heads (512 of the
1024 channels) for one batch element: Q/K/V projections, scores, softmax
(ACT exp with accumulated row sums), the [S,S] per-head attention probabilities
(bf16, written to DRAM - this is the 1 GiB `attn` output), a transposed DMA
readback feeding the attn @ V matmul, and the partial output projection.
Host sums the two half-channel partial projections per batch and adds bo.

Self-contained: builds the Bass/Tile program, runs it via
concourse.bass_utils.run_bass_kernel_spmd on cores 0-7, reassembles full
outputs. Returns (out[B,S,E] fp32, attn[B,H,S,S] fp32) like the reference.
"""

from contextlib import ExitStack

import numpy as np
import ml_dtypes

import concourse.bass as bass
import concourse.tile as tile
from concourse import mybir
from concourse import bass_utils

B, S, E, H, D = 4, 2048, 1024, 16, 64
BF16 = mybir.dt.bfloat16
F32 = mybir.dt.float32
NB = ml_dtypes.bfloat16
EXPFN = mybir.ActivationFunctionType.Exp

MAX_WAITS = 1


def _split_waits(nc):
    """walrus in this toolchain accepts ONE sync-wait per instruction; hoist
    extras onto same-engine NOPs inserted just before the owner (waiting
    earlier on the engine stream is safe under Tile's global schedule)."""
    for f in nc.m.functions:
        for b in f.blocks:
            insts = b.instructions  # live list
            i = 0
            while i < len(insts):
                inst = insts[i]
                si = inst.sync_info
                if si is not None and si.on_wait and len(si.on_wait) > MAX_WAITS:
                    waits = list(si.on_wait)
                    si.on_wait = waits[:MAX_WAITS]
                    for j, w in enumerate(waits[MAX_WAITS:]):
                        nop = mybir.InstNoOp(
                            name=f"{inst.name}_waitsplit{j}",
                            engine=inst.engine,
                            sync_info=mybir.SyncInfo(on_wait=[w], on_update=[]),
                        )
                        insts.insert(i, nop)
                        i += 1
                i += 1


def build_core(num_devices=8):
    nc = bass.Bass("TRN2", target_bir_lowering=False, debug=False,
                   num_devices=num_devices)
    d = {}
    for nm in ("xqT", "xkT", "xvT"):
        d[nm] = nc.dram_tensor(nm, [128, 8, S], BF16, kind="ExternalInput").ap()
    for nm in ("wqT", "wkT", "wvT"):
        d[nm] = nc.dram_tensor(nm, [128, 8, 512], BF16, kind="ExternalInput").ap()
    d["woT"] = nc.dram_tensor("woT", [128, 4, 1024], BF16, kind="ExternalInput").ap()
    d["bq2"] = nc.dram_tensor("bq2", [128, 4], F32, kind="ExternalInput").ap()
    d["bk2"] = nc.dram_tensor("bk2", [128, 4], F32, kind="ExternalInput").ap()
    d["bv2"] = nc.dram_tensor("bv2", [128, 512], F32, kind="ExternalInput").ap()
    attn_d = [
        nc.dram_tensor(f"attn{h}", [S, S], BF16, kind="ExternalOutput").ap()
        for h in range(8)
    ]
    outp = nc.dram_tensor("outp", [S, 1024], F32, kind="ExternalOutput").ap()

    NQ = S // 128
    NT = S // 128
    NS = S // 512

    with tile.TileContext(nc) as tc, ExitStack() as top:
        per = top.enter_context(tc.tile_pool(name="per", bufs=1))
        work = top.enter_context(tc.tile_pool(name="work", bufs=3))

        w_sb = {}
        for nm in ("wqT", "wkT", "wvT"):
            w_sb[nm] = per.tile([128, 8, 512], BF16, tag=nm, name=f"w_{nm}")
            nc.sync.dma_start(w_sb[nm][:], d[nm])
        woT_sb = per.tile([128, 4, 1024], BF16, tag="woT")
        nc.sync.dma_start(woT_sb[:], d["woT"])
        bq_sb = per.tile([128, 4], F32, tag="bq2")
        nc.sync.dma_start(bq_sb[:], d["bq2"])
        bk_sb = per.tile([128, 4], F32, tag="bk2")
        nc.sync.dma_start(bk_sb[:], d["bk2"])
        bv_sb = per.tile([128, 512], F32, tag="bv2")
        nc.sync.dma_start(bv_sb[:], d["bv2"])

        qt_all = per.tile([128, 4, S], BF16, tag="qt_all")
        kt_all = per.tile([128, 4, S], BF16, tag="kt_all")
        v_all = per.tile([128, NT, 512], BF16, tag="v_all")
        ctxT = per.tile([128, 4, S], BF16, tag="ctxT")

        # ---- projections: QT/KT in [d, q] layout, V in [k, d] layout
        with tc.tile_pool(name="xt", bufs=1) as xt_pool, \
             tc.tile_pool(name="ps_pj", bufs=4, space="PSUM") as ps_pj:
            for nm, wt, bias in (("xqT", "wqT", bq_sb), ("xkT", "wkT", bk_sb),
                                 ("xvT", "wvT", None)):
                x_sb = xt_pool.tile([128, 8, S], BF16, tag="xt")
                nc.sync.dma_start(x_sb[:], d[nm])
                if nm == "xvT":
                    for kt in range(NT):
                        pp = ps_pj.tile([128, 512], F32, tag="pj")
                        for eo in range(8):
                            nc.tensor.matmul(
                                pp[:], x_sb[:, eo, kt * 128:(kt + 1) * 128],
                                w_sb[wt][:, eo, :],
                                start=(eo == 0), stop=(eo == 7))
                        nc.vector.tensor_tensor(
                            v_all[:, kt, :], pp[:], bv_sb[:],
                            mybir.AluOpType.add)
                else:
                    dst = qt_all if nm == "xqT" else kt_all
                    for pair in range(4):
                        for qs in range(NS):
                            pp = ps_pj.tile([128, 512], F32, tag="pj")
                            for eo in range(8):
                                nc.tensor.matmul(
                                    pp[:],
                                    w_sb[wt][:, eo, pair * 128:(pair + 1) * 128],
                                    x_sb[:, eo, qs * 512:(qs + 1) * 512],
                                    start=(eo == 0), stop=(eo == 7))
                            nc.vector.tensor_scalar(
                                dst[:, pair, qs * 512:(qs + 1) * 512], pp[:],
                                bias[:, pair:pair + 1],
                                None, mybir.AluOpType.add)

        # ---- attention
        STRIP = 1024
        NHALF = S // STRIP
        with tc.tile_pool(name="ps_sc", bufs=2, space="PSUM") as ps_sc, \
             tc.tile_pool(name="ps_ctx", bufs=1, space="PSUM") as ps_ctx:

            def gen_A(pair):
                pending = []
                for qt in range(NQ):
                    for w in pending:
                        w()
                    pending = []
                    for hh in range(2):
                        h = pair * 2 + hh
                        hp = hh * 64
                        expA = work.tile([128, S], BF16, tag="expA", bufs=6,
                                         name=f"expA_{h}_{qt}")
                        zacc = work.tile([128, NHALF], F32, tag="zacc", bufs=6,
                                         name=f"zacc_{h}_{qt}")
                        for half in range(NHALF):
                            scores = ps_sc.tile([128, STRIP], F32, tag="sc",
                                                name=f"sc_{h}_{qt}_{half}")
                            for ki in range(STRIP // 512):
                                k0 = half * STRIP + ki * 512
                                nc.tensor.matmul(
                                    scores[:, ki * 512:(ki + 1) * 512],
                                    qt_all[hp:hp + 64, pair,
                                           qt * 128:(qt + 1) * 128],
                                    kt_all[hp:hp + 64, pair, k0:k0 + 512],
                                    start=True, stop=True)
                            nc.scalar.activation(
                                expA[:, half * STRIP:(half + 1) * STRIP],
                                scores[:], EXPFN, scale=0.125,
                                accum_out=zacc[:, half:half + 1])
                        rz = work.tile([128, 1], F32, tag="rz", bufs=6,
                                       name=f"rz_{h}_{qt}")
                        nc.vector.tensor_tensor(rz[:], zacc[:, 0:1],
                                                zacc[:, 1:2],
                                                mybir.AluOpType.add)
                        nc.vector.reciprocal(rz[:], rz[:])
                        nc.vector.tensor_scalar_mul(expA[:], expA[:], rz[:])
                        pending.append(
                            lambda h=h, qt=qt, expA=expA: nc.sync.dma_start(
                                attn_d[h][qt * 128:(qt + 1) * 128, :], expA[:]))
                    yield
                for w in pending:
                    w()

            PREFETCH = 2

            def gen_ctx(pair):
                psc = ps_ctx.tile([128, S], F32, tag="ctx", name=f"psc_{pair}")
                ats = {}
                for step in range(NT + PREFETCH):
                    if step < NT:
                        for hh in range(2):
                            h = pair * 2 + hh
                            at = work.tile([128, S], BF16, tag="at", bufs=8,
                                           name=f"at_{h}_{step}")
                            nc.sync.dma_start_transpose(
                                at[:],
                                attn_d[h][:, step * 128:(step + 1) * 128])
                            ats[(step, hh)] = at
                    kt = step - PREFETCH
                    if kt >= 0:
                        for hh in range(2):
                            at = ats.pop((kt, hh))
                            for qs in range(NS):
                                nc.tensor.matmul(
                                    psc[hh * 64:(hh + 1) * 64,
                                        qs * 512:(qs + 1) * 512],
                                    v_all[:, kt,
                                          pair * 128 + hh * 64:pair * 128 + hh * 64 + 64],
                                    at[:, qs * 512:(qs + 1) * 512],
                                    start=(kt == 0), stop=(kt == NT - 1),
                                    tile_position=(0, hh * 64),
                                    skip_group_check=True)
                    yield
                for qs in range(NS):
                    nc.vector.tensor_copy(ctxT[:, pair, qs * 512:(qs + 1) * 512],
                                          psc[:, qs * 512:(qs + 1) * 512])
                yield

            prev_ctx = None
            for pair in range(4):
                a = gen_A(pair)
                for _ in a:
                    if prev_ctx is not None:
                        next(prev_ctx, None)
                if prev_ctx is not None:
                    for _ in prev_ctx:
                        pass
                prev_ctx = gen_ctx(pair)
            for _ in prev_ctx:
                pass

        # ---- output projection (partial: our 512 channels of the contraction)
        with tc.tile_pool(name="ps_out", bufs=4, space="PSUM") as ps_out:
            for qt in range(NQ):
                for n in range(2):
                    pso = ps_out.tile([128, 512], F32, tag="out")
                    for co in range(4):
                        nc.tensor.matmul(
                            pso[:], ctxT[:, co, qt * 128:(qt + 1) * 128],
                            woT_sb[:, co, n * 512:(n + 1) * 512],
                            start=(co == 0), stop=(co == 3))
                    osb = work.tile([128, 512], F32, tag="osb")
                    nc.vector.tensor_copy(osb[:], pso[:])
                    nc.sync.dma_start(
                        outp[qt * 128:(qt + 1) * 128, n * 512:(n + 1) * 512],
                        osb[:])

    return nc


def _pack_core_inputs(query, key_in, value, Wq, bq, Wk, bk, Wv, bv, Wo, bo):
    """Per-core input dicts for cores 0..7 (core = b*2 + g)."""
    def xT_pack(a):  # [S, E] -> [128, 8, S] bf16
        return np.ascontiguousarray(
            a.T.reshape(8, 128, S).transpose(1, 0, 2).astype(NB))

    def wT_pack(w_rows):  # [512, E] -> [128, 8, 512] bf16
        return np.ascontiguousarray(
            w_rows.T.reshape(8, 128, 512).transpose(1, 0, 2).astype(NB))

    xq = [xT_pack(query[b]) for b in range(B)]
    xk = [xT_pack(key_in[b]) for b in range(B)]
    xv = [xT_pack(value[b]) for b in range(B)]
    per_g = []
    for g in range(2):
        cols = slice(g * 512, (g + 1) * 512)
        per_g.append({
            "wqT": wT_pack(Wq[cols, :]),
            "wkT": wT_pack(Wk[cols, :]),
            "wvT": wT_pack(Wv[cols, :]),
            "woT": np.ascontiguousarray(
                Wo[:, cols].T.reshape(4, 128, 1024).transpose(1, 0, 2).astype(NB)),
            "bq2": np.ascontiguousarray(
                bq[cols].reshape(4, 128).T.astype(np.float32)),
            "bk2": np.ascontiguousarray(
                bk[cols].reshape(4, 128).T.astype(np.float32)),
            "bv2": np.ascontiguousarray(
                np.tile(bv[cols].reshape(1, 512), (128, 1)).astype(np.float32)),
        })
    in_maps = []
    for b in range(B):
        for g in range(2):
            m = {"xqT": xq[b], "xkT": xk[b], "xvT": xv[b]}
            m.update(per_g[g])
            in_maps.append(m)
    return in_maps


_CACHED = {}


def kernel(query, key_in, value, Wq, bq, Wk, bk, Wv, bv, Wo, bo):
    query = np.asarray(query, np.float32)
    key_in = np.asarray(key_in, np.float32)
    value = np.asarray(value, np.float32)
    Wq, bq = np.asarray(Wq, np.float32), np.asarray(bq, np.float32)
    Wk, bk = np.asarray(Wk, np.float32), np.asarray(bk, np.float32)
    Wv, bv = np.asarray(Wv, np.float32), np.asarray(bv, np.float32)
    Wo, bo = np.asarray(Wo, np.float32), np.asarray(bo, np.float32)

    if "nc" not in _CACHED:
        nc = build_core(num_devices=8)
        _split_waits(nc)
        _CACHED["nc"] = nc
    nc = _CACHED["nc"]

    in_maps = _pack_core_inputs(query, key_in, value, Wq, bq, Wk, bk, Wv, bv,
                                Wo, bo)
    r = bass_utils.run_bass_kernel_spmd(nc, in_maps, core_ids=list(range(8)))

    out = np.zeros((B, S, E), np.float32)
    attn = np.zeros((B, H, S, S), np.float32)
    for b in range(B):
        for g in range(2):
            res = r.results[b * 2 + g]
            out[b] += res["outp"]
            for h in range(8):
                attn[b, g * 8 + h] = res[f"attn{h}"].astype(np.float32)
        out[b] += bo
    return out, attn


# revision 2
# speedup vs baseline: 1.2582x; 1.2582x over previous
"""MultiHeadAttention (B=4, S=2048, E=1024, H=16, D=64) on 8 Trainium2 cores.

Sharding: core = batch*2 + head_group. Each core owns one batch element and 8
heads (512 of 1024 channels): Q/K/V projections (bf16 PE matmuls), per-head
scores -> PSUM, softmax via ACT exp (scale=1/8, accumulated row sums), DVE
reciprocal+normalize, per-head [S,S] bf16 attention written to DRAM (the 1 GiB
`attn` output), transposed DMA-xbar readback feeding the attn @ V matmul
(head-pair col-tiled on the PE), and a partial output projection over its 512
contraction channels. Host sums the two partials per batch and adds bo.

Self-contained: builds the Bass/Tile program, runs it through
concourse.bass_utils.run_bass_kernel_spmd on cores 0-7, and reassembles the
full (out, attn) pair in the reference layout/dtypes.
"""

from contextlib import ExitStack

import numpy as np
import ml_dtypes

import concourse.bass as bass
import concourse.tile as tile
from concourse import mybir
from concourse import bass_utils

B, S, E, H, D = 4, 2048, 1024, 16, 64
BF16 = mybir.dt.bfloat16
F32 = mybir.dt.float32
NB = ml_dtypes.bfloat16
EXPFN = mybir.ActivationFunctionType.Exp

MAX_WAITS = 1


def _split_waits(nc):
    """The walrus build here accepts ONE sync-wait per instruction; hoist
    extras onto same-engine NOPs placed immediately before the owner (waiting
    earlier on the same engine stream is safe under Tile's global schedule)."""
    for f in nc.m.functions:
        for b in f.blocks:
            insts = b.instructions  # live list
            i = 0
            while i < len(insts):
                inst = insts[i]
                si = inst.sync_info
                if si is not None and si.on_wait and len(si.on_wait) > MAX_WAITS:
                    waits = list(si.on_wait)
                    si.on_wait = waits[:MAX_WAITS]
                    for j, w in enumerate(waits[MAX_WAITS:]):
                        nop = mybir.InstNoOp(
                            name=f"{inst.name}_waitsplit{j}",
                            engine=inst.engine,
                            sync_info=mybir.SyncInfo(on_wait=[w], on_update=[]),
                        )
                        insts.insert(i, nop)
                        i += 1
                i += 1


def build_core(num_devices=8):
    NQ, NT, NS = S // 128, S // 128, S // 512
    nc = bass.Bass("TRN2", target_bir_lowering=False, debug=False,
                   num_devices=num_devices)
    d = {}
    for nm in ("xqT", "xkT", "xvT"):
        d[nm] = nc.dram_tensor(nm, [128, 8, S], BF16, kind="ExternalInput").ap()
    for nm in ("wqT", "wkT", "wvT"):
        d[nm] = nc.dram_tensor(nm, [128, 8, 512], BF16, kind="ExternalInput").ap()
    d["woT"] = nc.dram_tensor("woT", [128, 4, 1024], BF16, kind="ExternalInput").ap()
    d["bq2"] = nc.dram_tensor("bq2", [128, 4], F32, kind="ExternalInput").ap()
    d["bk2"] = nc.dram_tensor("bk2", [128, 4], F32, kind="ExternalInput").ap()
    d["bv2"] = nc.dram_tensor("bv2", [128, 512], F32, kind="ExternalInput").ap()
    attn_d = [
        nc.dram_tensor(f"attn{h}", [S, S], BF16, kind="ExternalOutput").ap()
        for h in range(8)
    ]
    outp = nc.dram_tensor("outp", [S, 1024], F32, kind="ExternalOutput").ap()

    with tile.TileContext(nc) as tc, ExitStack() as top:
        per = top.enter_context(tc.tile_pool(name="per", bufs=1))
        work = top.enter_context(tc.tile_pool(name="work", bufs=3))

        w_sb = {}
        for nm in ("wqT", "wkT", "wvT"):
            w_sb[nm] = per.tile([128, 8, 512], BF16, tag=nm, name=f"w_{nm}")
            nc.sync.dma_start(w_sb[nm][:], d[nm])
        woT_sb = per.tile([128, 4, 1024], BF16, tag="woT")
        nc.sync.dma_start(woT_sb[:], d["woT"])
        bq_sb = per.tile([128, 4], F32, tag="bq2")
        nc.sync.dma_start(bq_sb[:], d["bq2"])
        bk_sb = per.tile([128, 4], F32, tag="bk2")
        nc.sync.dma_start(bk_sb[:], d["bk2"])
        bv_sb = per.tile([128, 512], F32, tag="bv2")
        nc.sync.dma_start(bv_sb[:], d["bv2"])

        qt_all = per.tile([128, 4, S], BF16, tag="qt_all")
        kt_all = per.tile([128, 4, S], BF16, tag="kt_all")
        v_all = per.tile([128, NT, 512], BF16, tag="v_all")
        ctxT = per.tile([128, 4, S], BF16, tag="ctxT")

        # ---- Q/K projections ([d, q] layout, head-pair packed on partitions)
        with tc.tile_pool(name="xt", bufs=1) as xt_pool, \
             tc.tile_pool(name="ps_pj", bufs=4, space="PSUM") as ps_pj:
            for nm, wt, bias in (("xqT", "wqT", bq_sb), ("xkT", "wkT", bk_sb)):
                x_sb = xt_pool.tile([128, 8, S], BF16, tag="xt", name=f"x_{nm}")
                nc.sync.dma_start(x_sb[:], d[nm])
                dst = qt_all if nm == "xqT" else kt_all
                for pair in range(4):
                    for qs in range(NS):
                        pp = ps_pj.tile([128, 512], F32, tag="pj",
                                        name=f"pj_{nm}_{pair}_{qs}")
                        for eo in range(8):
                            nc.tensor.matmul(
                                pp[:],
                                w_sb[wt][:, eo, pair * 128:(pair + 1) * 128],
                                x_sb[:, eo, qs * 512:(qs + 1) * 512],
                                start=(eo == 0), stop=(eo == 7))
                        nc.vector.tensor_scalar(
                            dst[:, pair, qs * 512:(qs + 1) * 512], pp[:],
                            bias[:, pair:pair + 1],
                            None, mybir.AluOpType.add)

        # ---- attention
        STRIP = 1024
        NHALF = S // STRIP
        with tc.tile_pool(name="ps_sc", bufs=2, space="PSUM") as ps_sc:
            v_stack = ExitStack()
            ps_v = v_stack.enter_context(
                tc.tile_pool(name="ps_v", bufs=2, space="PSUM"))
            ctx_stack = ExitStack()
            ps_ctx = None

            def gen_V():
                # V projection ([k, d] layout) streamed during phase A(0);
                # first consumed by ctx of pair 0, a full phase later
                for part in range(NT // 4):
                    xv_part = work.tile([128, 8, 512], BF16, tag="xvp", bufs=2,
                                        name=f"xvp_{part}")
                    nc.sync.dma_start(
                        xv_part[:], d["xvT"][:, :, part * 512:(part + 1) * 512])
                    for kt in range(part * 4, part * 4 + 4):
                        pp = ps_v.tile([128, 512], F32, tag="pjv", bufs=2,
                                       name=f"pjv_{kt}")
                        for eo in range(8):
                            nc.tensor.matmul(
                                pp[:],
                                xv_part[:, eo, (kt % 4) * 128:(kt % 4 + 1) * 128],
                                w_sb["wvT"][:, eo, :],
                                start=(eo == 0), stop=(eo == 7))
                        nc.vector.tensor_tensor(
                            v_all[:, kt, :], pp[:], bv_sb[:],
                            mybir.AluOpType.add)
                    yield

            def gen_A(pair):
                # attn writes are emitted one step late so the SP trigger
                # never waits on the still-running normalize chain
                pending = []
                for qt in range(NQ):
                    for w in pending:
                        w()
                    pending = []
                    for hh in range(2):
                        h = pair * 2 + hh
                        hp = hh * 64
                        expA = work.tile([128, S], BF16, tag="expA", bufs=5,
                                         name=f"expA_{h}_{qt}")
                        zacc = work.tile([128, NHALF], F32, tag="zacc", bufs=6,
                                         name=f"zacc_{h}_{qt}")
                        for half in range(NHALF):
                            scores = ps_sc.tile([128, STRIP], F32, tag="sc",
                                                name=f"sc_{h}_{qt}_{half}")
                            for ki in range(STRIP // 512):
                                k0 = half * STRIP + ki * 512
                                nc.tensor.matmul(
                                    scores[:, ki * 512:(ki + 1) * 512],
                                    qt_all[hp:hp + 64, pair,
                                           qt * 128:(qt + 1) * 128],
                                    kt_all[hp:hp + 64, pair, k0:k0 + 512],
                                    start=True, stop=True)
                            nc.scalar.activation(
                                expA[:, half * STRIP:(half + 1) * STRIP],
                                scores[:], EXPFN, scale=0.125,
                                accum_out=zacc[:, half:half + 1])
                        rz = work.tile([128, 1], F32, tag="rz", bufs=6,
                                       name=f"rz_{h}_{qt}")
                        nc.vector.tensor_tensor(rz[:], zacc[:, 0:1],
                                                zacc[:, 1:2],
                                                mybir.AluOpType.add)
                        nc.vector.reciprocal(rz[:], rz[:])
                        nc.vector.tensor_scalar_mul(expA[:], expA[:], rz[:])
                        pending.append(
                            lambda h=h, qt=qt, expA=expA: nc.sync.dma_start(
                                attn_d[h][qt * 128:(qt + 1) * 128, :], expA[:]))
                    yield
                for w in pending:
                    w()

            PREFETCH = 2

            def gen_ctx(pair):
                psc = ps_ctx.tile([128, S], F32, tag="ctx", name=f"psc_{pair}")
                ats = {}
                for step in range(NT + PREFETCH):
                    if step < NT:
                        for hh in range(2):
                            h = pair * 2 + hh
                            at = work.tile([128, S], BF16, tag="at", bufs=6,
                                           name=f"at_{h}_{step}")
                            nc.sync.dma_start_transpose(
                                at[:],
                                attn_d[h][:, step * 128:(step + 1) * 128])
                            ats[(step, hh)] = at
                    kt = step - PREFETCH
                    if kt >= 0:
                        for hh in range(2):
                            at = ats.pop((kt, hh))
                            for qs in range(NS):
                                nc.tensor.matmul(
                                    psc[hh * 64:(hh + 1) * 64,
                                        qs * 512:(qs + 1) * 512],
                                    v_all[:, kt,
                                          pair * 128 + hh * 64:pair * 128 + hh * 64 + 64],
                                    at[:, qs * 512:(qs + 1) * 512],
                                    start=(kt == 0), stop=(kt == NT - 1),
                                    tile_position=(0, hh * 64),
                                    skip_group_check=True)
                    yield
                for qs in range(NS):
                    nc.vector.tensor_copy(ctxT[:, pair, qs * 512:(qs + 1) * 512],
                                          psc[:, qs * 512:(qs + 1) * 512])
                yield

            def finish_v(vgen):
                nonlocal ps_ctx
                for _ in vgen:
                    pass
                v_stack.close()
                ps_ctx = ctx_stack.enter_context(
                    tc.tile_pool(name="ps_ctx", bufs=1, space="PSUM"))

            vgen = gen_V()
            for pair in range(4):
                for i, _ in enumerate(gen_A(pair)):
                    if pair == 0 and i % 4 == 0:
                        next(vgen, None)
                if pair == 0:
                    finish_v(vgen)
                for _ in gen_ctx(pair):
                    pass
            ctx_stack.close()

        # ---- output projection (partial over this core's 512 channels)
        with tc.tile_pool(name="ps_out", bufs=4, space="PSUM") as ps_out:
            for qt in range(NQ):
                for n in range(2):
                    pso = ps_out.tile([128, 512], F32, tag="out",
                                      name=f"pso_{qt}_{n}")
                    for co in range(4):
                        nc.tensor.matmul(
                            pso[:], ctxT[:, co, qt * 128:(qt + 1) * 128],
                            woT_sb[:, co, n * 512:(n + 1) * 512],
                            start=(co == 0), stop=(co == 3))
                    osb = work.tile([128, 512], F32, tag="osb",
                                    name=f"osb_{qt}_{n}")
                    nc.vector.tensor_copy(osb[:], pso[:])
                    nc.sync.dma_start(
                        outp[qt * 128:(qt + 1) * 128, n * 512:(n + 1) * 512],
                        osb[:])

    return nc


def _pack_core_inputs(query, key_in, value, Wq, bq, Wk, bk, Wv, bv, Wo, bo):
    """Per-core input dicts for cores 0..7 (core = b*2 + g)."""
    def xT_pack(a):  # [S, E] -> [128, 8, S] bf16
        return np.ascontiguousarray(
            a.T.reshape(8, 128, S).transpose(1, 0, 2).astype(NB))

    def wT_pack(w_rows):  # [512, E] -> [128, 8, 512] bf16
        return np.ascontiguousarray(
            w_rows.T.reshape(8, 128, 512).transpose(1, 0, 2).astype(NB))

    xq = [xT_pack(query[b]) for b in range(B)]
    xk = [xT_pack(key_in[b]) for b in range(B)]
    xv = [xT_pack(value[b]) for b in range(B)]
    per_g = []
    for g in range(2):
        cols = slice(g * 512, (g + 1) * 512)
        per_g.append({
            "wqT": wT_pack(Wq[cols, :]),
            "wkT": wT_pack(Wk[cols, :]),
            "wvT": wT_pack(Wv[cols, :]),
            "woT": np.ascontiguousarray(
                Wo[:, cols].T.reshape(4, 128, 1024).transpose(1, 0, 2).astype(NB)),
            "bq2": np.ascontiguousarray(
                bq[cols].reshape(4, 128).T.astype(np.float32)),
            "bk2": np.ascontiguousarray(
                bk[cols].reshape(4, 128).T.astype(np.float32)),
            "bv2": np.ascontiguousarray(
                np.tile(bv[cols].reshape(1, 512), (128, 1)).astype(np.float32)),
        })
    in_maps = []
    for b in range(B):
        for g in range(2):
            m = {"xqT": xq[b], "xkT": xk[b], "xvT": xv[b]}
            m.update(per_g[g])
            in_maps.append(m)
    return in_maps


_CACHED = {}


def kernel(query, key_in, value, Wq, bq, Wk, bk, Wv, bv, Wo, bo):
    query = np.asarray(query, np.float32)
    key_in = np.asarray(key_in, np.float32)
    value = np.asarray(value, np.float32)
    Wq, bq = np.asarray(Wq, np.float32), np.asarray(bq, np.float32)
    Wk, bk = np.asarray(Wk, np.float32), np.asarray(bk, np.float32)
    Wv, bv = np.asarray(Wv, np.float32), np.asarray(bv, np.float32)
    Wo, bo = np.asarray(Wo, np.float32), np.asarray(bo, np.float32)

    if "nc" not in _CACHED:
        nc = build_core(num_devices=8)
        _split_waits(nc)
        _CACHED["nc"] = nc
    nc = _CACHED["nc"]

    in_maps = _pack_core_inputs(query, key_in, value, Wq, bq, Wk, bk, Wv, bv,
                                Wo, bo)
    r = bass_utils.run_bass_kernel_spmd(nc, in_maps, core_ids=list(range(8)))

    out = np.zeros((B, S, E), np.float32)
    attn = np.zeros((B, H, S, S), np.float32)
    for b in range(B):
        for g in range(2):
            res = r.results[b * 2 + g]
            out[b] += res["outp"]
            for h in range(8):
                attn[b, g * 8 + h] = res[f"attn{h}"].astype(np.float32)
        out[b] += bo
    return out, attn


# revision 3
# speedup vs baseline: 1.2956x; 1.0297x over previous
"""MultiHeadAttention (B=4, S=2048, E=1024, H=16, D=64) on 8 Trainium2 cores.

Sharding: core = batch*2 + head_group. Each core owns one batch element and 8
heads (512 of 1024 channels): Q/K/V projections (bf16 PE matmuls), per-head
scores -> PSUM, softmax via ACT exp (scale=1/8, accumulated row sums), DVE
reciprocal+normalize, per-head [S,S] bf16 attention written to DRAM (the 1 GiB
`attn` output), transposed DMA-xbar readback feeding the attn @ V matmul
(head-pair col-tiled on the PE), and a partial output projection over its 512
contraction channels. Host sums the two partials per batch and adds bo.

Self-contained: builds the Bass/Tile program, runs it through
concourse.bass_utils.run_bass_kernel_spmd on cores 0-7, and reassembles the
full (out, attn) pair in the reference layout/dtypes.
"""

from contextlib import ExitStack

import numpy as np
import ml_dtypes

import concourse.bass as bass
import concourse.tile as tile
from concourse import mybir
from concourse import bass_utils

B, S, E, H, D = 4, 2048, 1024, 16, 64
BF16 = mybir.dt.bfloat16
F32 = mybir.dt.float32
NB = ml_dtypes.bfloat16
EXPFN = mybir.ActivationFunctionType.Exp

MAX_WAITS = 1


def _split_waits(nc):
    """The walrus build here accepts ONE sync-wait per instruction; hoist
    extras onto same-engine NOPs placed immediately before the owner (waiting
    earlier on the same engine stream is safe under Tile's global schedule)."""
    for f in nc.m.functions:
        for b in f.blocks:
            insts = b.instructions  # live list
            i = 0
            while i < len(insts):
                inst = insts[i]
                si = inst.sync_info
                if si is not None and si.on_wait and len(si.on_wait) > MAX_WAITS:
                    waits = list(si.on_wait)
                    si.on_wait = waits[:MAX_WAITS]
                    for j, w in enumerate(waits[MAX_WAITS:]):
                        nop = mybir.InstNoOp(
                            name=f"{inst.name}_waitsplit{j}",
                            engine=inst.engine,
                            sync_info=mybir.SyncInfo(on_wait=[w], on_update=[]),
                        )
                        insts.insert(i, nop)
                        i += 1
                i += 1


def build_core(num_devices=8):
    NQ, NT, NS = S // 128, S // 128, S // 512
    nc = bass.Bass("TRN2", target_bir_lowering=False, debug=False,
                   num_devices=num_devices)
    d = {}
    for nm in ("xqT", "xkT", "xvT"):
        d[nm] = nc.dram_tensor(nm, [128, 8, S], BF16, kind="ExternalInput").ap()
    for nm in ("wqT", "wkT", "wvT"):
        d[nm] = nc.dram_tensor(nm, [128, 8, 512], BF16, kind="ExternalInput").ap()
    d["woT"] = nc.dram_tensor("woT", [128, 4, 1024], BF16, kind="ExternalInput").ap()
    d["bq2"] = nc.dram_tensor("bq2", [128, 4], F32, kind="ExternalInput").ap()
    d["bk2"] = nc.dram_tensor("bk2", [128, 4], F32, kind="ExternalInput").ap()
    d["bv2"] = nc.dram_tensor("bv2", [128, 512], F32, kind="ExternalInput").ap()
    attn_d = [
        nc.dram_tensor(f"attn{h}", [S, S], BF16, kind="ExternalOutput").ap()
        for h in range(8)
    ]
    outp = nc.dram_tensor("outp", [S, 1024], F32, kind="ExternalOutput").ap()

    with tile.TileContext(nc) as tc, ExitStack() as top:
        per = top.enter_context(tc.tile_pool(name="per", bufs=1))
        work = top.enter_context(tc.tile_pool(name="work", bufs=3))

        w_sb = {}
        for nm in ("wqT", "wkT", "wvT"):
            w_sb[nm] = per.tile([128, 8, 512], BF16, tag=nm, name=f"w_{nm}")
            nc.sync.dma_start(w_sb[nm][:], d[nm])
        woT_sb = per.tile([128, 4, 1024], BF16, tag="woT")
        nc.sync.dma_start(woT_sb[:], d["woT"])
        bq_sb = per.tile([128, 4], F32, tag="bq2")
        nc.sync.dma_start(bq_sb[:], d["bq2"])
        bk_sb = per.tile([128, 4], F32, tag="bk2")
        nc.sync.dma_start(bk_sb[:], d["bk2"])
        bv_sb = per.tile([128, 512], F32, tag="bv2")
        nc.sync.dma_start(bv_sb[:], d["bv2"])

        qt_all = per.tile([128, 4, S], BF16, tag="qt_all")
        kt_all = per.tile([128, 4, S], BF16, tag="kt_all")
        v_all = per.tile([128, NT, 512], BF16, tag="v_all")
        ctxT = per.tile([128, 4, S], BF16, tag="ctxT")

        # ---- Q/K projections ([d, q] layout, head-pair packed on partitions)
        with tc.tile_pool(name="ps_pj", bufs=8, space="PSUM") as ps_pj:
            for qs in range(NS):
                for nm, wt, bias, dst in (("xqT", "wqT", bq_sb, qt_all),
                                          ("xkT", "wkT", bk_sb, kt_all)):
                    xp = work.tile([128, 8, 512], BF16, tag="xqk", bufs=3,
                                   name=f"xp_{nm}_{qs}")
                    nc.sync.dma_start(
                        xp[:], d[nm][:, :, qs * 512:(qs + 1) * 512])
                    for pair in range(4):
                        pp = ps_pj.tile([128, 512], F32, tag="pj",
                                        name=f"pj_{nm}_{pair}_{qs}")
                        for eo in range(8):
                            nc.tensor.matmul(
                                pp[:],
                                w_sb[wt][:, eo, pair * 128:(pair + 1) * 128],
                                xp[:, eo, :],
                                start=(eo == 0), stop=(eo == 7))
                        nc.vector.tensor_scalar(
                            dst[:, pair, qs * 512:(qs + 1) * 512], pp[:],
                            bias[:, pair:pair + 1],
                            None, mybir.AluOpType.add)

        # ---- attention
        STRIP = 1024
        NHALF = S // STRIP
        with tc.tile_pool(name="ps_sc", bufs=2, space="PSUM") as ps_sc:
            v_stack = ExitStack()
            ps_v = v_stack.enter_context(
                tc.tile_pool(name="ps_v", bufs=2, space="PSUM"))
            ctx_stack = ExitStack()
            ps_ctx = None

            def gen_V():
                # V projection ([k, d] layout) streamed during phase A(0);
                # first consumed by ctx of pair 0, a full phase later
                for part in range(NT // 4):
                    xv_part = work.tile([128, 8, 512], BF16, tag="xvp", bufs=2,
                                        name=f"xvp_{part}")
                    nc.sync.dma_start(
                        xv_part[:], d["xvT"][:, :, part * 512:(part + 1) * 512])
                    for kt in range(part * 4, part * 4 + 4):
                        pp = ps_v.tile([128, 512], F32, tag="pjv", bufs=2,
                                       name=f"pjv_{kt}")
                        for eo in range(8):
                            nc.tensor.matmul(
                                pp[:],
                                xv_part[:, eo, (kt % 4) * 128:(kt % 4 + 1) * 128],
                                w_sb["wvT"][:, eo, :],
                                start=(eo == 0), stop=(eo == 7))
                        nc.vector.tensor_tensor(
                            v_all[:, kt, :], pp[:], bv_sb[:],
                            mybir.AluOpType.add)
                    yield

            def gen_A(pair):
                # attn writes are emitted one step late so the SP trigger
                # never waits on the still-running normalize chain
                pending = []
                for qt in range(NQ):
                    for w in pending:
                        w()
                    pending = []
                    for hh in range(2):
                        h = pair * 2 + hh
                        hp = hh * 64
                        expA = work.tile([128, S], BF16, tag="expA", bufs=5,
                                         name=f"expA_{h}_{qt}")
                        zacc = work.tile([128, NHALF], F32, tag="zacc", bufs=6,
                                         name=f"zacc_{h}_{qt}")
                        for half in range(NHALF):
                            scores = ps_sc.tile([128, STRIP], F32, tag="sc",
                                                name=f"sc_{h}_{qt}_{half}")
                            for ki in range(STRIP // 512):
                                k0 = half * STRIP + ki * 512
                                nc.tensor.matmul(
                                    scores[:, ki * 512:(ki + 1) * 512],
                                    qt_all[hp:hp + 64, pair,
                                           qt * 128:(qt + 1) * 128],
                                    kt_all[hp:hp + 64, pair, k0:k0 + 512],
                                    start=True, stop=True)
                            nc.scalar.activation(
                                expA[:, half * STRIP:(half + 1) * STRIP],
                                scores[:], EXPFN, scale=0.125,
                                accum_out=zacc[:, half:half + 1])
                        rz = work.tile([128, 1], F32, tag="rz", bufs=6,
                                       name=f"rz_{h}_{qt}")
                        nc.vector.tensor_tensor(rz[:], zacc[:, 0:1],
                                                zacc[:, 1:2],
                                                mybir.AluOpType.add)
                        nc.vector.reciprocal(rz[:], rz[:])
                        nc.vector.tensor_scalar_mul(expA[:], expA[:], rz[:])
                        pending.append(
                            lambda h=h, qt=qt, expA=expA: nc.sync.dma_start(
                                attn_d[h][qt * 128:(qt + 1) * 128, :], expA[:]))
                    yield
                for w in pending:
                    w()

            PREFETCH = 2

            def gen_ctx(pair):
                psc = ps_ctx.tile([128, S], F32, tag="ctx", name=f"psc_{pair}")
                ats = {}
                for step in range(NT + PREFETCH):
                    if step < NT:
                        for hh in range(2):
                            h = pair * 2 + hh
                            at = work.tile([128, S], BF16, tag="at", bufs=6,
                                           name=f"at_{h}_{step}")
                            nc.sync.dma_start_transpose(
                                at[:],
                                attn_d[h][:, step * 128:(step + 1) * 128])
                            ats[(step, hh)] = at
                    kt = step - PREFETCH
                    if kt >= 0:
                        for hh in range(2):
                            at = ats.pop((kt, hh))
                            for qs in range(NS):
                                nc.tensor.matmul(
                                    psc[hh * 64:(hh + 1) * 64,
                                        qs * 512:(qs + 1) * 512],
                                    v_all[:, kt,
                                          pair * 128 + hh * 64:pair * 128 + hh * 64 + 64],
                                    at[:, qs * 512:(qs + 1) * 512],
                                    start=(kt == 0), stop=(kt == NT - 1),
                                    tile_position=(0, hh * 64),
                                    skip_group_check=True)
                    yield
                for qs in range(NS):
                    nc.vector.tensor_copy(ctxT[:, pair, qs * 512:(qs + 1) * 512],
                                          psc[:, qs * 512:(qs + 1) * 512])
                yield

            def finish_v(vgen):
                nonlocal ps_ctx
                for _ in vgen:
                    pass
                v_stack.close()
                ps_ctx = ctx_stack.enter_context(
                    tc.tile_pool(name="ps_ctx", bufs=1, space="PSUM"))

            vgen = gen_V()
            for pair in range(4):
                for i, _ in enumerate(gen_A(pair)):
                    if pair == 0 and i % 4 == 0:
                        next(vgen, None)
                if pair == 0:
                    finish_v(vgen)
                for _ in gen_ctx(pair):
                    pass
            ctx_stack.close()

        # ---- output projection (partial over this core's 512 channels)
        with tc.tile_pool(name="ps_out", bufs=4, space="PSUM") as ps_out:
            for qt in range(NQ):
                for n in range(2):
                    pso = ps_out.tile([128, 512], F32, tag="out",
                                      name=f"pso_{qt}_{n}")
                    for co in range(4):
                        nc.tensor.matmul(
                            pso[:], ctxT[:, co, qt * 128:(qt + 1) * 128],
                            woT_sb[:, co, n * 512:(n + 1) * 512],
                            start=(co == 0), stop=(co == 3))
                    osb = work.tile([128, 512], F32, tag="osb",
                                    name=f"osb_{qt}_{n}")
                    nc.vector.tensor_copy(osb[:], pso[:])
                    nc.sync.dma_start(
                        outp[qt * 128:(qt + 1) * 128, n * 512:(n + 1) * 512],
                        osb[:])

    return nc


def _pack_core_inputs(query, key_in, value, Wq, bq, Wk, bk, Wv, bv, Wo, bo):
    """Per-core input dicts for cores 0..7 (core = b*2 + g)."""
    def xT_pack(a):  # [S, E] -> [128, 8, S] bf16
        return np.ascontiguousarray(
            a.T.reshape(8, 128, S).transpose(1, 0, 2).astype(NB))

    def wT_pack(w_rows):  # [512, E] -> [128, 8, 512] bf16
        return np.ascontiguousarray(
            w_rows.T.reshape(8, 128, 512).transpose(1, 0, 2).astype(NB))

    xq = [xT_pack(query[b]) for b in range(B)]
    xk = [xT_pack(key_in[b]) for b in range(B)]
    xv = [xT_pack(value[b]) for b in range(B)]
    per_g = []
    for g in range(2):
        cols = slice(g * 512, (g + 1) * 512)
        per_g.append({
            "wqT": wT_pack(Wq[cols, :]),
            "wkT": wT_pack(Wk[cols, :]),
            "wvT": wT_pack(Wv[cols, :]),
            "woT": np.ascontiguousarray(
                Wo[:, cols].T.reshape(4, 128, 1024).transpose(1, 0, 2).astype(NB)),
            "bq2": np.ascontiguousarray(
                bq[cols].reshape(4, 128).T.astype(np.float32)),
            "bk2": np.ascontiguousarray(
                bk[cols].reshape(4, 128).T.astype(np.float32)),
            "bv2": np.ascontiguousarray(
                np.tile(bv[cols].reshape(1, 512), (128, 1)).astype(np.float32)),
        })
    in_maps = []
    for b in range(B):
        for g in range(2):
            m = {"xqT": xq[b], "xkT": xk[b], "xvT": xv[b]}
            m.update(per_g[g])
            in_maps.append(m)
    return in_maps


_CACHED = {}


def kernel(query, key_in, value, Wq, bq, Wk, bk, Wv, bv, Wo, bo):
    query = np.asarray(query, np.float32)
    key_in = np.asarray(key_in, np.float32)
    value = np.asarray(value, np.float32)
    Wq, bq = np.asarray(Wq, np.float32), np.asarray(bq, np.float32)
    Wk, bk = np.asarray(Wk, np.float32), np.asarray(bk, np.float32)
    Wv, bv = np.asarray(Wv, np.float32), np.asarray(bv, np.float32)
    Wo, bo = np.asarray(Wo, np.float32), np.asarray(bo, np.float32)

    if "nc" not in _CACHED:
        nc = build_core(num_devices=8)
        _split_waits(nc)
        _CACHED["nc"] = nc
    nc = _CACHED["nc"]

    in_maps = _pack_core_inputs(query, key_in, value, Wq, bq, Wk, bk, Wv, bv,
                                Wo, bo)
    r = bass_utils.run_bass_kernel_spmd(nc, in_maps, core_ids=list(range(8)))

    out = np.zeros((B, S, E), np.float32)
    attn = np.zeros((B, H, S, S), np.float32)
    for b in range(B):
        for g in range(2):
            res = r.results[b * 2 + g]
            out[b] += res["outp"]
            for h in range(8):
                attn[b, g * 8 + h] = res[f"attn{h}"].astype(np.float32)
        out[b] += bo
    return out, attn


# revision 4
# speedup vs baseline: 1.3207x; 1.0193x over previous
"""MultiHeadAttention (B=4, S=2048, E=1024, H=16, D=64) on 8 Trainium2 cores.

Sharding: core = batch*2 + head_group. Each core owns one batch element and 8
heads (512 of 1024 channels): Q/K/V projections (bf16 PE matmuls), per-head
scores -> PSUM, softmax via ACT exp (scale=1/8, accumulated row sums), DVE
reciprocal+normalize, per-head [S,S] bf16 attention written to DRAM (the 1 GiB
`attn` output), transposed DMA-xbar readback feeding the attn @ V matmul
(head-pair col-tiled on the PE), and a partial output projection over its 512
contraction channels. Host sums the two partials per batch and adds bo.

Self-contained: builds the Bass/Tile program, runs it through
concourse.bass_utils.run_bass_kernel_spmd on cores 0-7, and reassembles the
full (out, attn) pair in the reference layout/dtypes.
"""

from contextlib import ExitStack

import numpy as np
import ml_dtypes

import concourse.bass as bass
import concourse.tile as tile
from concourse import mybir
from concourse import bass_utils

B, S, E, H, D = 4, 2048, 1024, 16, 64
BF16 = mybir.dt.bfloat16
F32 = mybir.dt.float32
NB = ml_dtypes.bfloat16
EXPFN = mybir.ActivationFunctionType.Exp

MAX_WAITS = 1


def _split_waits(nc):
    """The walrus build here accepts ONE sync-wait per instruction; hoist
    extras onto same-engine NOPs placed immediately before the owner (waiting
    earlier on the same engine stream is safe under Tile's global schedule)."""
    for f in nc.m.functions:
        for b in f.blocks:
            insts = b.instructions  # live list
            i = 0
            while i < len(insts):
                inst = insts[i]
                si = inst.sync_info
                if si is not None and si.on_wait and len(si.on_wait) > MAX_WAITS:
                    waits = list(si.on_wait)
                    si.on_wait = waits[:MAX_WAITS]
                    for j, w in enumerate(waits[MAX_WAITS:]):
                        nop = mybir.InstNoOp(
                            name=f"{inst.name}_waitsplit{j}",
                            engine=inst.engine,
                            sync_info=mybir.SyncInfo(on_wait=[w], on_update=[]),
                        )
                        insts.insert(i, nop)
                        i += 1
                i += 1


def build_core(num_devices=8):
    NQ, NT, NS = S // 128, S // 128, S // 512
    nc = bass.Bass("TRN2", target_bir_lowering=False, debug=False,
                   num_devices=num_devices)
    d = {}
    for nm in ("xqT", "xkT", "xvT"):
        d[nm] = nc.dram_tensor(nm, [128, 8, S], BF16, kind="ExternalInput").ap()
    for nm in ("wqT", "wkT", "wvT"):
        d[nm] = nc.dram_tensor(nm, [128, 8, 512], BF16, kind="ExternalInput").ap()
    d["woT"] = nc.dram_tensor("woT", [128, 4, 1024], BF16, kind="ExternalInput").ap()
    d["bq2"] = nc.dram_tensor("bq2", [128, 4], F32, kind="ExternalInput").ap()
    d["bk2"] = nc.dram_tensor("bk2", [128, 4], F32, kind="ExternalInput").ap()
    d["bv2"] = nc.dram_tensor("bv2", [128, 512], F32, kind="ExternalInput").ap()
    attn_d = [
        nc.dram_tensor(f"attn{h}", [S, S], BF16, kind="ExternalOutput").ap()
        for h in range(8)
    ]
    outp = nc.dram_tensor("outp", [S, 1024], F32, kind="ExternalOutput").ap()

    with tile.TileContext(nc) as tc, ExitStack() as top:
        per = top.enter_context(tc.tile_pool(name="per", bufs=1))
        work = top.enter_context(tc.tile_pool(name="work", bufs=3))

        w_sb = {}
        for nm in ("wqT", "wkT", "wvT"):
            w_sb[nm] = per.tile([128, 8, 512], BF16, tag=nm, name=f"w_{nm}")
            nc.sync.dma_start(w_sb[nm][:], d[nm])
        woT_sb = per.tile([128, 4, 1024], BF16, tag="woT")
        nc.sync.dma_start(woT_sb[:], d["woT"])
        bq_sb = per.tile([128, 4], F32, tag="bq2")
        nc.sync.dma_start(bq_sb[:], d["bq2"])
        bk_sb = per.tile([128, 4], F32, tag="bk2")
        nc.sync.dma_start(bk_sb[:], d["bk2"])
        bv_sb = per.tile([128, 512], F32, tag="bv2")
        nc.sync.dma_start(bv_sb[:], d["bv2"])

        qt_all = per.tile([128, 4, S], BF16, tag="qt_all")
        kt_all = per.tile([128, 4, S], BF16, tag="kt_all")
        v_all = per.tile([128, NT, 512], BF16, tag="v_all")
        ctxT = per.tile([128, 4, S], BF16, tag="ctxT")

        # ---- Q/K projections ([d, q] layout, head-pair packed on partitions)
        with tc.tile_pool(name="ps_pj", bufs=8, space="PSUM") as ps_pj:
            for qs in range(NS):
                for nm, wt, bias, dst in (("xqT", "wqT", bq_sb, qt_all),
                                          ("xkT", "wkT", bk_sb, kt_all)):
                    xp = work.tile([128, 8, 512], BF16, tag="xqk", bufs=3,
                                   name=f"xp_{nm}_{qs}")
                    nc.sync.dma_start(
                        xp[:], d[nm][:, :, qs * 512:(qs + 1) * 512])
                    for pair in range(4):
                        pp = ps_pj.tile([128, 512], F32, tag="pj",
                                        name=f"pj_{nm}_{pair}_{qs}")
                        for eo in range(8):
                            nc.tensor.matmul(
                                pp[:],
                                w_sb[wt][:, eo, pair * 128:(pair + 1) * 128],
                                xp[:, eo, :],
                                start=(eo == 0), stop=(eo == 7))
                        nc.vector.tensor_scalar(
                            dst[:, pair, qs * 512:(qs + 1) * 512], pp[:],
                            bias[:, pair:pair + 1],
                            None, mybir.AluOpType.add)

        # ---- attention
        STRIP = 1024
        NHALF = S // STRIP
        with tc.tile_pool(name="ps_sc", bufs=2, space="PSUM") as ps_sc:
            v_stack = ExitStack()
            ps_v = v_stack.enter_context(
                tc.tile_pool(name="ps_v", bufs=2, space="PSUM"))
            ctx_stack = ExitStack()
            ps_ctx = None

            def gen_V():
                # V projection ([k, d] layout) streamed during phase A(0);
                # first consumed by ctx of pair 0, a full phase later
                for part in range(NT // 4):
                    xv_part = work.tile([128, 8, 512], BF16, tag="xvp", bufs=2,
                                        name=f"xvp_{part}")
                    nc.sync.dma_start(
                        xv_part[:], d["xvT"][:, :, part * 512:(part + 1) * 512])
                    for kt in range(part * 4, part * 4 + 4):
                        pp = ps_v.tile([128, 512], F32, tag="pjv", bufs=2,
                                       name=f"pjv_{kt}")
                        for eo in range(8):
                            nc.tensor.matmul(
                                pp[:],
                                xv_part[:, eo, (kt % 4) * 128:(kt % 4 + 1) * 128],
                                w_sb["wvT"][:, eo, :],
                                start=(eo == 0), stop=(eo == 7))
                        nc.vector.tensor_tensor(
                            v_all[:, kt, :], pp[:], bv_sb[:],
                            mybir.AluOpType.add)
                    yield

            def gen_A(pair):
                # attn writes are emitted one step late so the SP trigger
                # never waits on the still-running normalize chain
                pending = []
                for qt in range(NQ):
                    for w in pending:
                        w()
                    pending = []
                    for hh in range(2):
                        h = pair * 2 + hh
                        hp = hh * 64
                        expA = work.tile([128, S], BF16, tag="expA", bufs=5,
                                         name=f"expA_{h}_{qt}")
                        zacc = work.tile([128, NHALF], F32, tag="zacc", bufs=6,
                                         name=f"zacc_{h}_{qt}")
                        for half in range(NHALF):
                            scores = ps_sc.tile([128, STRIP], F32, tag="sc",
                                                name=f"sc_{h}_{qt}_{half}")
                            for ki in range(STRIP // 512):
                                k0 = half * STRIP + ki * 512
                                nc.tensor.matmul(
                                    scores[:, ki * 512:(ki + 1) * 512],
                                    qt_all[hp:hp + 64, pair,
                                           qt * 128:(qt + 1) * 128],
                                    kt_all[hp:hp + 64, pair, k0:k0 + 512],
                                    start=True, stop=True)
                            nc.scalar.activation(
                                expA[:, half * STRIP:(half + 1) * STRIP],
                                scores[:], EXPFN, scale=0.125,
                                accum_out=zacc[:, half:half + 1])
                        rz = work.tile([128, 1], F32, tag="rz", bufs=6,
                                       name=f"rz_{h}_{qt}")
                        nc.vector.tensor_tensor(rz[:], zacc[:, 0:1],
                                                zacc[:, 1:2],
                                                mybir.AluOpType.add)
                        nc.vector.reciprocal(rz[:], rz[:])
                        nc.vector.tensor_scalar_mul(expA[:], expA[:], rz[:])
                        pending.append(
                            lambda h=h, qt=qt, expA=expA: nc.sync.dma_start(
                                attn_d[h][qt * 128:(qt + 1) * 128, :], expA[:]))
                    yield
                for w in pending:
                    w()

            PREFETCH = 2

            def gen_ctx(pair):
                psc = ps_ctx.tile([128, S], F32, tag="ctx", name=f"psc_{pair}")
                ats = {}
                for step in range(NT + PREFETCH):
                    if step < NT:
                        for hh in range(2):
                            h = pair * 2 + hh
                            at = work.tile([128, S], BF16, tag="at", bufs=6,
                                           name=f"at_{h}_{step}")
                            nc.sync.dma_start_transpose(
                                at[:],
                                attn_d[h][:, step * 128:(step + 1) * 128])
                            ats[(step, hh)] = at
                    kt = step - PREFETCH
                    if kt >= 0:
                        for hh in range(2):
                            at = ats.pop((kt, hh))
                            for qs in range(NS):
                                nc.tensor.matmul(
                                    psc[hh * 64:(hh + 1) * 64,
                                        qs * 512:(qs + 1) * 512],
                                    v_all[:, kt,
                                          pair * 128 + hh * 64:pair * 128 + hh * 64 + 64],
                                    at[:, qs * 512:(qs + 1) * 512],
                                    start=(kt == 0), stop=(kt == NT - 1),
                                    tile_position=(0, hh * 64),
                                    skip_group_check=True)
                    yield
                for qs in range(NS):
                    nc.vector.tensor_copy(ctxT[:, pair, qs * 512:(qs + 1) * 512],
                                          psc[:, qs * 512:(qs + 1) * 512])
                yield

            def finish_v(vgen):
                nonlocal ps_ctx
                for _ in vgen:
                    pass
                v_stack.close()
                ps_ctx = ctx_stack.enter_context(
                    tc.tile_pool(name="ps_ctx", bufs=1, space="PSUM"))

            vgen = gen_V()
            for pair in range(4):
                for i, _ in enumerate(gen_A(pair)):
                    if pair == 0 and i % 4 == 0:
                        next(vgen, None)
                if pair == 0:
                    finish_v(vgen)
                for _ in gen_ctx(pair):
                    pass
            ctx_stack.close()

        # ---- output projection (partial over this core's 512 channels)
        with tc.tile_pool(name="ps_out", bufs=4, space="PSUM") as ps_out:
            for qt in range(NQ):
                for n in range(2):
                    pso = ps_out.tile([128, 512], F32, tag="out",
                                      name=f"pso_{qt}_{n}")
                    for co in range(4):
                        nc.tensor.matmul(
                            pso[:], ctxT[:, co, qt * 128:(qt + 1) * 128],
                            woT_sb[:, co, n * 512:(n + 1) * 512],
                            start=(co == 0), stop=(co == 3))
                    osb = work.tile([128, 512], F32, tag="osb", bufs=6,
                                    name=f"osb_{qt}_{n}")
                    nc.vector.tensor_copy(osb[:], pso[:])
                    nc.sync.dma_start(
                        outp[qt * 128:(qt + 1) * 128, n * 512:(n + 1) * 512],
                        osb[:])

    return nc


def _pack_core_inputs(query, key_in, value, Wq, bq, Wk, bk, Wv, bv, Wo, bo):
    """Per-core input dicts for cores 0..7 (core = b*2 + g)."""
    def xT_pack(a):  # [S, E] -> [128, 8, S] bf16
        return np.ascontiguousarray(
            a.T.reshape(8, 128, S).transpose(1, 0, 2).astype(NB))

    def wT_pack(w_rows):  # [512, E] -> [128, 8, 512] bf16
        return np.ascontiguousarray(
            w_rows.T.reshape(8, 128, 512).transpose(1, 0, 2).astype(NB))

    xq = [xT_pack(query[b]) for b in range(B)]
    xk = [xT_pack(key_in[b]) for b in range(B)]
    xv = [xT_pack(value[b]) for b in range(B)]
    per_g = []
    for g in range(2):
        cols = slice(g * 512, (g + 1) * 512)
        per_g.append({
            "wqT": wT_pack(Wq[cols, :]),
            "wkT": wT_pack(Wk[cols, :]),
            "wvT": wT_pack(Wv[cols, :]),
            "woT": np.ascontiguousarray(
                Wo[:, cols].T.reshape(4, 128, 1024).transpose(1, 0, 2).astype(NB)),
            "bq2": np.ascontiguousarray(
                bq[cols].reshape(4, 128).T.astype(np.float32)),
            "bk2": np.ascontiguousarray(
                bk[cols].reshape(4, 128).T.astype(np.float32)),
            "bv2": np.ascontiguousarray(
                np.tile(bv[cols].reshape(1, 512), (128, 1)).astype(np.float32)),
        })
    in_maps = []
    for b in range(B):
        for g in range(2):
            m = {"xqT": xq[b], "xkT": xk[b], "xvT": xv[b]}
            m.update(per_g[g])
            in_maps.append(m)
    return in_maps


_CACHED = {}


def kernel(query, key_in, value, Wq, bq, Wk, bk, Wv, bv, Wo, bo):
    query = np.asarray(query, np.float32)
    key_in = np.asarray(key_in, np.float32)
    value = np.asarray(value, np.float32)
    Wq, bq = np.asarray(Wq, np.float32), np.asarray(bq, np.float32)
    Wk, bk = np.asarray(Wk, np.float32), np.asarray(bk, np.float32)
    Wv, bv = np.asarray(Wv, np.float32), np.asarray(bv, np.float32)
    Wo, bo = np.asarray(Wo, np.float32), np.asarray(bo, np.float32)

    if "nc" not in _CACHED:
        nc = build_core(num_devices=8)
        _split_waits(nc)
        _CACHED["nc"] = nc
    nc = _CACHED["nc"]

    in_maps = _pack_core_inputs(query, key_in, value, Wq, bq, Wk, bk, Wv, bv,
                                Wo, bo)
    r = bass_utils.run_bass_kernel_spmd(nc, in_maps, core_ids=list(range(8)))

    out = np.zeros((B, S, E), np.float32)
    attn = np.zeros((B, H, S, S), np.float32)
    for b in range(B):
        for g in range(2):
            res = r.results[b * 2 + g]
            out[b] += res["outp"]
            for h in range(8):
                attn[b, g * 8 + h] = res[f"attn{h}"].astype(np.float32)
        out[b] += bo
    return out, attn


# revision 5
# speedup vs baseline: 1.3979x; 1.0585x over previous
"""MultiHeadAttention (B=4, S=2048, E=1024, H=16, D=64) on 8 Trainium2 cores.

Sharding: core = batch*2 + head_group. Each core owns one batch element and 8
heads (512 of 1024 channels): Q/K/V projections (bf16 PE matmuls), per-head
scores -> PSUM, softmax via ACT exp (scale=1/8, accumulated row sums), DVE
reciprocal+normalize, per-head [S,S] bf16 attention written to DRAM (the 1 GiB
`attn` output), transposed DMA-xbar readback feeding the attn @ V matmul
(head-pair col-tiled on the PE), and a partial output projection over its 512
contraction channels. Host sums the two partials per batch and adds bo.

Self-contained: builds the Bass/Tile program, runs it through
concourse.bass_utils.run_bass_kernel_spmd on cores 0-7, and reassembles the
full (out, attn) pair in the reference layout/dtypes.
"""

from contextlib import ExitStack

import numpy as np
import ml_dtypes

import concourse.bass as bass
import concourse.tile as tile
from concourse import mybir
from concourse import bass_utils

B, S, E, H, D = 4, 2048, 1024, 16, 64
BF16 = mybir.dt.bfloat16
F32 = mybir.dt.float32
NB = ml_dtypes.bfloat16
EXPFN = mybir.ActivationFunctionType.Exp

MAX_WAITS = 1


def _split_waits(nc):
    """The walrus build here accepts ONE sync-wait per instruction; hoist
    extras onto same-engine NOPs placed immediately before the owner (waiting
    earlier on the same engine stream is safe under Tile's global schedule)."""
    for f in nc.m.functions:
        for b in f.blocks:
            insts = b.instructions  # live list
            i = 0
            while i < len(insts):
                inst = insts[i]
                si = inst.sync_info
                if si is not None and si.on_wait and len(si.on_wait) > MAX_WAITS:
                    waits = list(si.on_wait)
                    si.on_wait = waits[:MAX_WAITS]
                    for j, w in enumerate(waits[MAX_WAITS:]):
                        nop = mybir.InstNoOp(
                            name=f"{inst.name}_waitsplit{j}",
                            engine=inst.engine,
                            sync_info=mybir.SyncInfo(on_wait=[w], on_update=[]),
                        )
                        insts.insert(i, nop)
                        i += 1
                i += 1


def build_core(num_devices=8):
    NQ, NT, NS = S // 128, S // 128, S // 512
    nc = bass.Bass("TRN2", target_bir_lowering=False, debug=False,
                   num_devices=num_devices)
    d = {}
    for nm in ("xqT", "xkT", "xvT"):
        d[nm] = nc.dram_tensor(nm, [128, 8, S], BF16, kind="ExternalInput").ap()
    for nm in ("wqT", "wkT", "wvT"):
        d[nm] = nc.dram_tensor(nm, [128, 8, 512], BF16, kind="ExternalInput").ap()
    d["woT"] = nc.dram_tensor("woT", [128, 4, 1024], BF16, kind="ExternalInput").ap()
    d["bq2"] = nc.dram_tensor("bq2", [128, 4], F32, kind="ExternalInput").ap()
    d["bk2"] = nc.dram_tensor("bk2", [128, 4], F32, kind="ExternalInput").ap()
    d["bv2"] = nc.dram_tensor("bv2", [128, 512], F32, kind="ExternalInput").ap()
    attn_d = [
        nc.dram_tensor(f"attn{h}", [S, S], BF16, kind="ExternalOutput").ap()
        for h in range(8)
    ]
    outp = nc.dram_tensor("outp", [S, 1024], F32, kind="ExternalOutput").ap()

    with tile.TileContext(nc) as tc, ExitStack() as top:
        per = top.enter_context(tc.tile_pool(name="per", bufs=1))
        work = top.enter_context(tc.tile_pool(name="work", bufs=3))

        w_sb = {}
        for nm in ("wqT", "wkT", "wvT"):
            w_sb[nm] = per.tile([128, 8, 512], BF16, tag=nm, name=f"w_{nm}")
            nc.sync.dma_start(w_sb[nm][:], d[nm])
        woT_sb = per.tile([128, 4, 1024], BF16, tag="woT")
        nc.sync.dma_start(woT_sb[:], d["woT"])
        bq_sb = per.tile([128, 4], F32, tag="bq2")
        nc.sync.dma_start(bq_sb[:], d["bq2"])
        bk_sb = per.tile([128, 4], F32, tag="bk2")
        nc.sync.dma_start(bk_sb[:], d["bk2"])
        bv_sb = per.tile([128, 512], F32, tag="bv2")
        nc.sync.dma_start(bv_sb[:], d["bv2"])

        qt_all = per.tile([128, 4, S], BF16, tag="qt_all")
        kt_all = per.tile([128, 4, S], BF16, tag="kt_all")
        v_all = per.tile([128, NT, 512], BF16, tag="v_all")
        ctxT = per.tile([128, 4, S], BF16, tag="ctxT")

        # ---- Q/K projections ([d, q] layout, head-pair packed on partitions)
        with tc.tile_pool(name="ps_pj", bufs=8, space="PSUM") as ps_pj:
            for qs in range(NS):
                for nm, wt, bias, dst in (("xqT", "wqT", bq_sb, qt_all),
                                          ("xkT", "wkT", bk_sb, kt_all)):
                    xp = work.tile([128, 8, 512], BF16, tag="xqk", bufs=3,
                                   name=f"xp_{nm}_{qs}")
                    nc.sync.dma_start(
                        xp[:], d[nm][:, :, qs * 512:(qs + 1) * 512])
                    for pair in range(4):
                        pp = ps_pj.tile([128, 512], F32, tag="pj",
                                        name=f"pj_{nm}_{pair}_{qs}")
                        for eo in range(8):
                            nc.tensor.matmul(
                                pp[:],
                                w_sb[wt][:, eo, pair * 128:(pair + 1) * 128],
                                xp[:, eo, :],
                                start=(eo == 0), stop=(eo == 7))
                        nc.vector.tensor_scalar(
                            dst[:, pair, qs * 512:(qs + 1) * 512], pp[:],
                            bias[:, pair:pair + 1],
                            None, mybir.AluOpType.add)

        # ---- attention
        STRIP = 1024
        NHALF = S // STRIP
        with tc.tile_pool(name="ps_sc", bufs=2, space="PSUM") as ps_sc:
            v_stack = ExitStack()
            ps_v = v_stack.enter_context(
                tc.tile_pool(name="ps_v", bufs=2, space="PSUM"))
            ctx_stack = ExitStack()
            ps_ctx = None

            def gen_V():
                # V projection ([k, d] layout) streamed during phase A(0);
                # first consumed by ctx of pair 0, a full phase later
                for part in range(NT // 4):
                    xv_part = work.tile([128, 8, 512], BF16, tag="xvp", bufs=2,
                                        name=f"xvp_{part}")
                    nc.sync.dma_start(
                        xv_part[:], d["xvT"][:, :, part * 512:(part + 1) * 512])
                    for kt in range(part * 4, part * 4 + 4):
                        pp = ps_v.tile([128, 512], F32, tag="pjv", bufs=2,
                                       name=f"pjv_{kt}")
                        for eo in range(8):
                            nc.tensor.matmul(
                                pp[:],
                                xv_part[:, eo, (kt % 4) * 128:(kt % 4 + 1) * 128],
                                w_sb["wvT"][:, eo, :],
                                start=(eo == 0), stop=(eo == 7))
                        nc.vector.tensor_tensor(
                            v_all[:, kt, :], pp[:], bv_sb[:],
                            mybir.AluOpType.add)
                    yield

            def gen_A(pair):
                # attn writes are emitted one step late so the SP trigger
                # never waits on the still-running normalize chain
                pending = []
                for qt in range(NQ):
                    for w in pending:
                        w()
                    pending = []
                    for hh in range(2):
                        h = pair * 2 + hh
                        hp = hh * 64
                        expA = work.tile([128, S], BF16, tag="expA", bufs=10,
                                         name=f"expA_{h}_{qt}")
                        zacc = work.tile([128, NHALF], F32, tag="zacc", bufs=6,
                                         name=f"zacc_{h}_{qt}")
                        for half in range(NHALF):
                            scores = ps_sc.tile([128, STRIP], F32, tag="sc",
                                                name=f"sc_{h}_{qt}_{half}")
                            for ki in range(STRIP // 512):
                                k0 = half * STRIP + ki * 512
                                nc.tensor.matmul(
                                    scores[:, ki * 512:(ki + 1) * 512],
                                    qt_all[hp:hp + 64, pair,
                                           qt * 128:(qt + 1) * 128],
                                    kt_all[hp:hp + 64, pair, k0:k0 + 512],
                                    start=True, stop=True)
                            nc.scalar.activation(
                                expA[:, half * STRIP:(half + 1) * STRIP],
                                scores[:], EXPFN, scale=0.125,
                                accum_out=zacc[:, half:half + 1])
                        rz = work.tile([128, 1], F32, tag="rz", bufs=6,
                                       name=f"rz_{h}_{qt}")
                        nc.vector.tensor_tensor(rz[:], zacc[:, 0:1],
                                                zacc[:, 1:2],
                                                mybir.AluOpType.add)
                        nc.vector.reciprocal(rz[:], rz[:])
                        nc.vector.tensor_scalar_mul(expA[:], expA[:], rz[:])
                        pending.append(
                            lambda h=h, qt=qt, expA=expA: nc.sync.dma_start(
                                attn_d[h][qt * 128:(qt + 1) * 128, :], expA[:]))
                    yield
                for w in pending:
                    w()

            PREFETCH = 2

            def gen_ctx(pair):
                psc = ps_ctx.tile([128, S], F32, tag="ctx", name=f"psc_{pair}")
                ats = {}
                for step in range(NT + PREFETCH):
                    if step < NT:
                        for hh in range(2):
                            h = pair * 2 + hh
                            at = work.tile([128, S], BF16, tag="at", bufs=4,
                                           name=f"at_{h}_{step}")
                            nc.sync.dma_start_transpose(
                                at[:],
                                attn_d[h][:, step * 128:(step + 1) * 128])
                            ats[(step, hh)] = at
                    kt = step - PREFETCH
                    if kt >= 0:
                        for hh in range(2):
                            at = ats.pop((kt, hh))
                            for qs in range(NS):
                                nc.tensor.matmul(
                                    psc[hh * 64:(hh + 1) * 64,
                                        qs * 512:(qs + 1) * 512],
                                    v_all[:, kt,
                                          pair * 128 + hh * 64:pair * 128 + hh * 64 + 64],
                                    at[:, qs * 512:(qs + 1) * 512],
                                    start=(kt == 0), stop=(kt == NT - 1),
                                    tile_position=(0, hh * 64),
                                    skip_group_check=True)
                    yield
                for qs in range(NS):
                    nc.vector.tensor_copy(ctxT[:, pair, qs * 512:(qs + 1) * 512],
                                          psc[:, qs * 512:(qs + 1) * 512])
                yield

            def finish_v(vgen):
                nonlocal ps_ctx
                for _ in vgen:
                    pass
                v_stack.close()
                ps_ctx = ctx_stack.enter_context(
                    tc.tile_pool(name="ps_ctx", bufs=1, space="PSUM"))

            vgen = gen_V()
            for pair in range(4):
                for i, _ in enumerate(gen_A(pair)):
                    if pair == 0 and i % 4 == 0:
                        next(vgen, None)
                if pair == 0:
                    finish_v(vgen)
                for _ in gen_ctx(pair):
                    pass
            ctx_stack.close()

        # ---- output projection (partial over this core's 512 channels)
        with tc.tile_pool(name="ps_out", bufs=4, space="PSUM") as ps_out:
            for qt in range(NQ):
                for n in range(2):
                    pso = ps_out.tile([128, 512], F32, tag="out",
                                      name=f"pso_{qt}_{n}")
                    for co in range(4):
                        nc.tensor.matmul(
                            pso[:], ctxT[:, co, qt * 128:(qt + 1) * 128],
                            woT_sb[:, co, n * 512:(n + 1) * 512],
                            start=(co == 0), stop=(co == 3))
                    osb = work.tile([128, 512], F32, tag="osb", bufs=6,
                                    name=f"osb_{qt}_{n}")
                    nc.vector.tensor_copy(osb[:], pso[:])
                    nc.sync.dma_start(
                        outp[qt * 128:(qt + 1) * 128, n * 512:(n + 1) * 512],
                        osb[:])

    return nc


def _pack_core_inputs(query, key_in, value, Wq, bq, Wk, bk, Wv, bv, Wo, bo):
    """Per-core input dicts for cores 0..7 (core = b*2 + g)."""
    def xT_pack(a):  # [S, E] -> [128, 8, S] bf16
        return np.ascontiguousarray(
            a.T.reshape(8, 128, S).transpose(1, 0, 2).astype(NB))

    def wT_pack(w_rows):  # [512, E] -> [128, 8, 512] bf16
        return np.ascontiguousarray(
            w_rows.T.reshape(8, 128, 512).transpose(1, 0, 2).astype(NB))

    xq = [xT_pack(query[b]) for b in range(B)]
    xk = [xT_pack(key_in[b]) for b in range(B)]
    xv = [xT_pack(value[b]) for b in range(B)]
    per_g = []
    for g in range(2):
        cols = slice(g * 512, (g + 1) * 512)
        per_g.append({
            "wqT": wT_pack(Wq[cols, :]),
            "wkT": wT_pack(Wk[cols, :]),
            "wvT": wT_pack(Wv[cols, :]),
            "woT": np.ascontiguousarray(
                Wo[:, cols].T.reshape(4, 128, 1024).transpose(1, 0, 2).astype(NB)),
            "bq2": np.ascontiguousarray(
                bq[cols].reshape(4, 128).T.astype(np.float32)),
            "bk2": np.ascontiguousarray(
                bk[cols].reshape(4, 128).T.astype(np.float32)),
            "bv2": np.ascontiguousarray(
                np.tile(bv[cols].reshape(1, 512), (128, 1)).astype(np.float32)),
        })
    in_maps = []
    for b in range(B):
        for g in range(2):
            m = {"xqT": xq[b], "xkT": xk[b], "xvT": xv[b]}
            m.update(per_g[g])
            in_maps.append(m)
    return in_maps


_CACHED = {}


def kernel(query, key_in, value, Wq, bq, Wk, bk, Wv, bv, Wo, bo):
    query = np.asarray(query, np.float32)
    key_in = np.asarray(key_in, np.float32)
    value = np.asarray(value, np.float32)
    Wq, bq = np.asarray(Wq, np.float32), np.asarray(bq, np.float32)
    Wk, bk = np.asarray(Wk, np.float32), np.asarray(bk, np.float32)
    Wv, bv = np.asarray(Wv, np.float32), np.asarray(bv, np.float32)
    Wo, bo = np.asarray(Wo, np.float32), np.asarray(bo, np.float32)

    if "nc" not in _CACHED:
        nc = build_core(num_devices=8)
        _split_waits(nc)
        _CACHED["nc"] = nc
    nc = _CACHED["nc"]

    in_maps = _pack_core_inputs(query, key_in, value, Wq, bq, Wk, bk, Wv, bv,
                                Wo, bo)
    r = bass_utils.run_bass_kernel_spmd(nc, in_maps, core_ids=list(range(8)))

    out = np.zeros((B, S, E), np.float32)
    attn = np.zeros((B, H, S, S), np.float32)
    for b in range(B):
        for g in range(2):
            res = r.results[b * 2 + g]
            out[b] += res["outp"]
            for h in range(8):
                attn[b, g * 8 + h] = res[f"attn{h}"].astype(np.float32)
        out[b] += bo
    return out, attn


# revision 6
# speedup vs baseline: 1.4398x; 1.0300x over previous
"""MultiHeadAttention (B=4, S=2048, E=1024, H=16, D=64) on 8 Trainium2 cores.

Sharding: core = batch*2 + head_group. Each core owns one batch element and 8
heads (512 of 1024 channels): Q/K/V projections (bf16 PE matmuls), per-head
scores -> PSUM, softmax via ACT exp (scale=1/8, accumulated row sums), DVE
reciprocal+normalize, per-head [S,S] bf16 attention written to DRAM (the 1 GiB
`attn` output), transposed DMA-xbar readback feeding the attn @ V matmul
(head-pair col-tiled on the PE), and a partial output projection over its 512
contraction channels. Host sums the two partials per batch and adds bo.

Self-contained: builds the Bass/Tile program, runs it through
concourse.bass_utils.run_bass_kernel_spmd on cores 0-7, and reassembles the
full (out, attn) pair in the reference layout/dtypes.
"""

from contextlib import ExitStack

import numpy as np
import ml_dtypes

import concourse.bass as bass
import concourse.tile as tile
from concourse import mybir
from concourse import bass_utils

B, S, E, H, D = 4, 2048, 1024, 16, 64
BF16 = mybir.dt.bfloat16
F32 = mybir.dt.float32
NB = ml_dtypes.bfloat16
EXPFN = mybir.ActivationFunctionType.Exp

MAX_WAITS = 1


def _split_waits(nc):
    """The walrus build here accepts ONE sync-wait per instruction; hoist
    extras onto same-engine NOPs placed immediately before the owner (waiting
    earlier on the same engine stream is safe under Tile's global schedule)."""
    for f in nc.m.functions:
        for b in f.blocks:
            insts = b.instructions  # live list
            i = 0
            while i < len(insts):
                inst = insts[i]
                si = inst.sync_info
                if si is not None and si.on_wait and len(si.on_wait) > MAX_WAITS:
                    waits = list(si.on_wait)
                    si.on_wait = waits[:MAX_WAITS]
                    for j, w in enumerate(waits[MAX_WAITS:]):
                        nop = mybir.InstNoOp(
                            name=f"{inst.name}_waitsplit{j}",
                            engine=inst.engine,
                            sync_info=mybir.SyncInfo(on_wait=[w], on_update=[]),
                        )
                        insts.insert(i, nop)
                        i += 1
                i += 1


def build_core(num_devices=8):
    NQ, NT, NS = S // 128, S // 128, S // 512
    nc = bass.Bass("TRN2", target_bir_lowering=False, debug=False,
                   num_devices=num_devices)
    d = {}
    for nm in ("xqT", "xkT", "xvT"):
        d[nm] = nc.dram_tensor(nm, [128, 8, S], BF16, kind="ExternalInput").ap()
    for nm in ("wqT", "wkT", "wvT"):
        d[nm] = nc.dram_tensor(nm, [128, 8, 512], BF16, kind="ExternalInput").ap()
    d["woT"] = nc.dram_tensor("woT", [128, 4, 1024], BF16, kind="ExternalInput").ap()
    d["bq2"] = nc.dram_tensor("bq2", [128, 4], F32, kind="ExternalInput").ap()
    d["bk2"] = nc.dram_tensor("bk2", [128, 4], F32, kind="ExternalInput").ap()
    d["bv2"] = nc.dram_tensor("bv2", [128, 512], F32, kind="ExternalInput").ap()
    attn_d = [
        nc.dram_tensor(f"attn{h}", [S, S], BF16, kind="ExternalOutput").ap()
        for h in range(8)
    ]
    outp = nc.dram_tensor("outp", [S, 1024], F32, kind="ExternalOutput").ap()

    with tile.TileContext(nc) as tc, ExitStack() as top:
        per = top.enter_context(tc.tile_pool(name="per", bufs=1))
        work = top.enter_context(tc.tile_pool(name="work", bufs=3))

        w_sb = {}
        for nm in ("wqT", "wkT", "wvT"):
            w_sb[nm] = per.tile([128, 8, 512], BF16, tag=nm, name=f"w_{nm}")
            nc.sync.dma_start(w_sb[nm][:], d[nm])
        woT_sb = per.tile([128, 4, 1024], BF16, tag="woT")
        nc.sync.dma_start(woT_sb[:], d["woT"])
        bq_sb = per.tile([128, 4], F32, tag="bq2")
        nc.sync.dma_start(bq_sb[:], d["bq2"])
        bk_sb = per.tile([128, 4], F32, tag="bk2")
        nc.sync.dma_start(bk_sb[:], d["bk2"])
        bv_sb = per.tile([128, 512], F32, tag="bv2")
        nc.sync.dma_start(bv_sb[:], d["bv2"])

        qt_all = per.tile([128, 4, S], BF16, tag="qt_all")
        kt_all = per.tile([128, 4, S], BF16, tag="kt_all")
        v_all = per.tile([128, NT, 512], BF16, tag="v_all")
        ctxT = per.tile([128, 4, S], BF16, tag="ctxT")

        # ---- Q/K projections ([d, q] layout, head-pair packed on partitions)
        with tc.tile_pool(name="ps_pj", bufs=8, space="PSUM") as ps_pj:
            for qs in range(NS):
                for nm, wt, bias, dst in (("xqT", "wqT", bq_sb, qt_all),
                                          ("xkT", "wkT", bk_sb, kt_all)):
                    xp = work.tile([128, 8, 512], BF16, tag="xqk", bufs=3,
                                   name=f"xp_{nm}_{qs}")
                    nc.sync.dma_start(
                        xp[:], d[nm][:, :, qs * 512:(qs + 1) * 512])
                    for pair in range(4):
                        pp = ps_pj.tile([128, 512], F32, tag="pj",
                                        name=f"pj_{nm}_{pair}_{qs}")
                        for eo in range(8):
                            nc.tensor.matmul(
                                pp[:],
                                w_sb[wt][:, eo, pair * 128:(pair + 1) * 128],
                                xp[:, eo, :],
                                start=(eo == 0), stop=(eo == 7))
                        nc.vector.tensor_scalar(
                            dst[:, pair, qs * 512:(qs + 1) * 512], pp[:],
                            bias[:, pair:pair + 1],
                            None, mybir.AluOpType.add)

        # ---- attention
        STRIP = 1024
        NHALF = S // STRIP
        with tc.tile_pool(name="ps_sc", bufs=2, space="PSUM") as ps_sc:
            v_stack = ExitStack()
            ps_v = v_stack.enter_context(
                tc.tile_pool(name="ps_v", bufs=2, space="PSUM"))
            ctx_stack = ExitStack()
            ps_ctx = None

            def gen_V():
                # V projection ([k, d] layout) streamed during phase A(0);
                # first consumed by ctx of pair 0, a full phase later
                for part in range(NT // 4):
                    xv_part = work.tile([128, 8, 512], BF16, tag="xvp", bufs=1,
                                        name=f"xvp_{part}")
                    nc.sync.dma_start(
                        xv_part[:], d["xvT"][:, :, part * 512:(part + 1) * 512])
                    for kt in range(part * 4, part * 4 + 4):
                        pp = ps_v.tile([128, 512], F32, tag="pjv", bufs=2,
                                       name=f"pjv_{kt}")
                        for eo in range(8):
                            nc.tensor.matmul(
                                pp[:],
                                xv_part[:, eo, (kt % 4) * 128:(kt % 4 + 1) * 128],
                                w_sb["wvT"][:, eo, :],
                                start=(eo == 0), stop=(eo == 7))
                        nc.vector.tensor_tensor(
                            v_all[:, kt, :], pp[:], bv_sb[:],
                            mybir.AluOpType.add)
                    yield

            def gen_A(pair):
                # attn writes are emitted one step late so the SP trigger
                # never waits on the still-running normalize chain
                pending = []
                for qt in range(NQ):
                    for w in pending:
                        w()
                    pending = []
                    for hh in range(2):
                        h = pair * 2 + hh
                        hp = hh * 64
                        expA = work.tile([128, S], BF16, tag="expA", bufs=13,
                                         name=f"expA_{h}_{qt}")
                        zacc = work.tile([128, NHALF], F32, tag="zacc", bufs=6,
                                         name=f"zacc_{h}_{qt}")
                        for half in range(NHALF):
                            scores = ps_sc.tile([128, STRIP], F32, tag="sc",
                                                name=f"sc_{h}_{qt}_{half}")
                            for ki in range(STRIP // 512):
                                k0 = half * STRIP + ki * 512
                                nc.tensor.matmul(
                                    scores[:, ki * 512:(ki + 1) * 512],
                                    qt_all[hp:hp + 64, pair,
                                           qt * 128:(qt + 1) * 128],
                                    kt_all[hp:hp + 64, pair, k0:k0 + 512],
                                    start=True, stop=True)
                            nc.scalar.activation(
                                expA[:, half * STRIP:(half + 1) * STRIP],
                                scores[:], EXPFN, scale=0.125,
                                accum_out=zacc[:, half:half + 1])
                        rz = work.tile([128, 1], F32, tag="rz", bufs=6,
                                       name=f"rz_{h}_{qt}")
                        nc.vector.tensor_tensor(rz[:], zacc[:, 0:1],
                                                zacc[:, 1:2],
                                                mybir.AluOpType.add)
                        nc.vector.reciprocal(rz[:], rz[:])
                        nc.vector.tensor_scalar_mul(expA[:], expA[:], rz[:])
                        pending.append(
                            lambda h=h, qt=qt, expA=expA: nc.sync.dma_start(
                                attn_d[h][qt * 128:(qt + 1) * 128, :], expA[:]))
                    yield
                for w in pending:
                    w()

            PREFETCH = 2

            def gen_ctx(pair):
                psc = ps_ctx.tile([128, S], F32, tag="ctx", name=f"psc_{pair}")
                ats = {}
                for step in range(NT + PREFETCH):
                    if step < NT:
                        for hh in range(2):
                            h = pair * 2 + hh
                            at = work.tile([128, S], BF16, tag="at", bufs=4,
                                           name=f"at_{h}_{step}")
                            nc.sync.dma_start_transpose(
                                at[:],
                                attn_d[h][:, step * 128:(step + 1) * 128])
                            ats[(step, hh)] = at
                    kt = step - PREFETCH
                    if kt >= 0:
                        for hh in range(2):
                            at = ats.pop((kt, hh))
                            for qs in range(NS):
                                nc.tensor.matmul(
                                    psc[hh * 64:(hh + 1) * 64,
                                        qs * 512:(qs + 1) * 512],
                                    v_all[:, kt,
                                          pair * 128 + hh * 64:pair * 128 + hh * 64 + 64],
                                    at[:, qs * 512:(qs + 1) * 512],
                                    start=(kt == 0), stop=(kt == NT - 1),
                                    tile_position=(0, hh * 64),
                                    skip_group_check=True)
                    yield
                for qs in range(NS):
                    nc.vector.tensor_copy(ctxT[:, pair, qs * 512:(qs + 1) * 512],
                                          psc[:, qs * 512:(qs + 1) * 512])
                yield

            def finish_v(vgen):
                nonlocal ps_ctx
                for _ in vgen:
                    pass
                v_stack.close()
                ps_ctx = ctx_stack.enter_context(
                    tc.tile_pool(name="ps_ctx", bufs=1, space="PSUM"))

            vgen = gen_V()
            for pair in range(4):
                for i, _ in enumerate(gen_A(pair)):
                    if pair == 0 and i % 4 == 0:
                        next(vgen, None)
                if pair == 0:
                    finish_v(vgen)
                for _ in gen_ctx(pair):
                    pass
            ctx_stack.close()

        # ---- output projection (partial over this core's 512 channels)
        with tc.tile_pool(name="ps_out", bufs=4, space="PSUM") as ps_out:
            for qt in range(NQ):
                for n in range(2):
                    pso = ps_out.tile([128, 512], F32, tag="out",
                                      name=f"pso_{qt}_{n}")
                    for co in range(4):
                        nc.tensor.matmul(
                            pso[:], ctxT[:, co, qt * 128:(qt + 1) * 128],
                            woT_sb[:, co, n * 512:(n + 1) * 512],
                            start=(co == 0), stop=(co == 3))
                    osb = work.tile([128, 512], F32, tag="osb", bufs=4,
                                    name=f"osb_{qt}_{n}")
                    nc.vector.tensor_copy(osb[:], pso[:])
                    nc.sync.dma_start(
                        outp[qt * 128:(qt + 1) * 128, n * 512:(n + 1) * 512],
                        osb[:])

    return nc


def _pack_core_inputs(query, key_in, value, Wq, bq, Wk, bk, Wv, bv, Wo, bo):
    """Per-core input dicts for cores 0..7 (core = b*2 + g)."""
    def xT_pack(a):  # [S, E] -> [128, 8, S] bf16
        return np.ascontiguousarray(
            a.T.reshape(8, 128, S).transpose(1, 0, 2).astype(NB))

    def wT_pack(w_rows):  # [512, E] -> [128, 8, 512] bf16
        return np.ascontiguousarray(
            w_rows.T.reshape(8, 128, 512).transpose(1, 0, 2).astype(NB))

    xq = [xT_pack(query[b]) for b in range(B)]
    xk = [xT_pack(key_in[b]) for b in range(B)]
    xv = [xT_pack(value[b]) for b in range(B)]
    per_g = []
    for g in range(2):
        cols = slice(g * 512, (g + 1) * 512)
        per_g.append({
            "wqT": wT_pack(Wq[cols, :]),
            "wkT": wT_pack(Wk[cols, :]),
            "wvT": wT_pack(Wv[cols, :]),
            "woT": np.ascontiguousarray(
                Wo[:, cols].T.reshape(4, 128, 1024).transpose(1, 0, 2).astype(NB)),
            "bq2": np.ascontiguousarray(
                bq[cols].reshape(4, 128).T.astype(np.float32)),
            "bk2": np.ascontiguousarray(
                bk[cols].reshape(4, 128).T.astype(np.float32)),
            "bv2": np.ascontiguousarray(
                np.tile(bv[cols].reshape(1, 512), (128, 1)).astype(np.float32)),
        })
    in_maps = []
    for b in range(B):
        for g in range(2):
            m = {"xqT": xq[b], "xkT": xk[b], "xvT": xv[b]}
            m.update(per_g[g])
            in_maps.append(m)
    return in_maps


_CACHED = {}


def kernel(query, key_in, value, Wq, bq, Wk, bk, Wv, bv, Wo, bo):
    query = np.asarray(query, np.float32)
    key_in = np.asarray(key_in, np.float32)
    value = np.asarray(value, np.float32)
    Wq, bq = np.asarray(Wq, np.float32), np.asarray(bq, np.float32)
    Wk, bk = np.asarray(Wk, np.float32), np.asarray(bk, np.float32)
    Wv, bv = np.asarray(Wv, np.float32), np.asarray(bv, np.float32)
    Wo, bo = np.asarray(Wo, np.float32), np.asarray(bo, np.float32)

    if "nc" not in _CACHED:
        nc = build_core(num_devices=8)
        _split_waits(nc)
        _CACHED["nc"] = nc
    nc = _CACHED["nc"]

    in_maps = _pack_core_inputs(query, key_in, value, Wq, bq, Wk, bk, Wv, bv,
                                Wo, bo)
    r = bass_utils.run_bass_kernel_spmd(nc, in_maps, core_ids=list(range(8)))

    out = np.zeros((B, S, E), np.float32)
    attn = np.zeros((B, H, S, S), np.float32)
    for b in range(B):
        for g in range(2):
            res = r.results[b * 2 + g]
            out[b] += res["outp"]
            for h in range(8):
                attn[b, g * 8 + h] = res[f"attn{h}"].astype(np.float32)
        out[b] += bo
    return out, attn


# revision 7
# speedup vs baseline: 1.4754x; 1.0247x over previous
"""MultiHeadAttention (B=4, S=2048, E=1024, H=16, D=64) on 8 Trainium2 cores.

Sharding: core = batch*2 + head_group. Each core owns one batch element and 8
heads (512 of 1024 channels): Q/K/V projections (bf16 PE matmuls), per-head
scores -> PSUM, softmax via ACT exp (scale=1/8, accumulated row sums), DVE
reciprocal+normalize, per-head [S,S] bf16 attention written to DRAM (the 1 GiB
`attn` output), transposed DMA-xbar readback feeding the attn @ V matmul
(head-pair col-tiled on the PE), and a partial output projection over its 512
contraction channels. Host sums the two partials per batch and adds bo.

Self-contained: builds the Bass/Tile program, runs it through
concourse.bass_utils.run_bass_kernel_spmd on cores 0-7, and reassembles the
full (out, attn) pair in the reference layout/dtypes.
"""

from contextlib import ExitStack

import numpy as np
import ml_dtypes

import concourse.bass as bass
import concourse.tile as tile
from concourse import mybir
from concourse import bass_utils

B, S, E, H, D = 4, 2048, 1024, 16, 64
BF16 = mybir.dt.bfloat16
F32 = mybir.dt.float32
NB = ml_dtypes.bfloat16
EXPFN = mybir.ActivationFunctionType.Exp

MAX_WAITS = 1


def _split_waits(nc):
    """The walrus build here accepts ONE sync-wait per instruction; hoist
    extras onto same-engine NOPs placed immediately before the owner (waiting
    earlier on the same engine stream is safe under Tile's global schedule)."""
    for f in nc.m.functions:
        for b in f.blocks:
            insts = b.instructions  # live list
            i = 0
            while i < len(insts):
                inst = insts[i]
                si = inst.sync_info
                if si is not None and si.on_wait and len(si.on_wait) > MAX_WAITS:
                    waits = list(si.on_wait)
                    si.on_wait = waits[:MAX_WAITS]
                    for j, w in enumerate(waits[MAX_WAITS:]):
                        nop = mybir.InstNoOp(
                            name=f"{inst.name}_waitsplit{j}",
                            engine=inst.engine,
                            sync_info=mybir.SyncInfo(on_wait=[w], on_update=[]),
                        )
                        insts.insert(i, nop)
                        i += 1
                i += 1


def build_core(num_devices=8):
    NQ, NT, NS = S // 128, S // 128, S // 512
    nc = bass.Bass("TRN2", target_bir_lowering=False, debug=False,
                   num_devices=num_devices)
    d = {}
    for nm in ("xqT", "xkT", "xvT"):
        d[nm] = nc.dram_tensor(nm, [128, 8, S], BF16, kind="ExternalInput").ap()
    for nm in ("wqT", "wkT", "wvT"):
        d[nm] = nc.dram_tensor(nm, [128, 8, 512], BF16, kind="ExternalInput").ap()
    d["woT"] = nc.dram_tensor("woT", [128, 4, 1024], BF16, kind="ExternalInput").ap()
    d["bq2"] = nc.dram_tensor("bq2", [128, 4], F32, kind="ExternalInput").ap()
    d["bk2"] = nc.dram_tensor("bk2", [128, 4], F32, kind="ExternalInput").ap()
    d["bv2"] = nc.dram_tensor("bv2", [128, 512], F32, kind="ExternalInput").ap()
    attn_d = [
        nc.dram_tensor(f"attn{h}", [S, S], BF16, kind="ExternalOutput").ap()
        for h in range(8)
    ]
    outp = nc.dram_tensor("outp", [S, 1024], F32, kind="ExternalOutput").ap()

    with tile.TileContext(nc) as tc, ExitStack() as top:
        per = top.enter_context(tc.tile_pool(name="per", bufs=1))
        work = top.enter_context(tc.tile_pool(name="work", bufs=3))

        w_sb = {}
        for nm in ("wqT", "wkT", "wvT"):
            w_sb[nm] = per.tile([128, 8, 512], BF16, tag=nm, name=f"w_{nm}")
            nc.sync.dma_start(w_sb[nm][:], d[nm])
        woT_sb = per.tile([128, 4, 1024], BF16, tag="woT")
        nc.sync.dma_start(woT_sb[:], d["woT"])
        bq_sb = per.tile([128, 4], F32, tag="bq2")
        nc.sync.dma_start(bq_sb[:], d["bq2"])
        bk_sb = per.tile([128, 4], F32, tag="bk2")
        nc.sync.dma_start(bk_sb[:], d["bk2"])
        bv_sb = per.tile([128, 512], F32, tag="bv2")
        nc.sync.dma_start(bv_sb[:], d["bv2"])

        qt_all = per.tile([128, 4, S], BF16, tag="qt_all")
        kt_all = per.tile([128, 4, S], BF16, tag="kt_all")
        v_all = per.tile([128, NT, 512], BF16, tag="v_all")
        ctxT = per.tile([128, 4, S], BF16, tag="ctxT")

        # ---- Q/K projections ([d, q] layout, head-pair packed on partitions)
        with tc.tile_pool(name="ps_pj", bufs=8, space="PSUM") as ps_pj:
            for qs in range(NS):
                for nm, wt, bias, dst in (("xqT", "wqT", bq_sb, qt_all),
                                          ("xkT", "wkT", bk_sb, kt_all)):
                    xp = work.tile([128, 8, 512], BF16, tag="xqk", bufs=2,
                                   name=f"xp_{nm}_{qs}")
                    nc.sync.dma_start(
                        xp[:], d[nm][:, :, qs * 512:(qs + 1) * 512])
                    for pair in range(4):
                        pp = ps_pj.tile([128, 512], F32, tag="pj",
                                        name=f"pj_{nm}_{pair}_{qs}")
                        for eo in range(8):
                            nc.tensor.matmul(
                                pp[:],
                                w_sb[wt][:, eo, pair * 128:(pair + 1) * 128],
                                xp[:, eo, :],
                                start=(eo == 0), stop=(eo == 7))
                        nc.vector.tensor_scalar(
                            dst[:, pair, qs * 512:(qs + 1) * 512], pp[:],
                            bias[:, pair:pair + 1],
                            None, mybir.AluOpType.add)

        # ---- attention
        STRIP = 1024
        NHALF = S // STRIP
        with tc.tile_pool(name="ps_sc", bufs=2, space="PSUM") as ps_sc:
            v_stack = ExitStack()
            ps_v = v_stack.enter_context(
                tc.tile_pool(name="ps_v", bufs=2, space="PSUM"))
            ctx_stack = ExitStack()
            ps_ctx = None

            def gen_V():
                # V projection ([k, d] layout) streamed during phase A(0);
                # first consumed by ctx of pair 0, a full phase later
                for part in range(NT // 4):
                    xv_part = work.tile([128, 8, 512], BF16, tag="xvp", bufs=1,
                                        name=f"xvp_{part}")
                    nc.sync.dma_start(
                        xv_part[:], d["xvT"][:, :, part * 512:(part + 1) * 512])
                    for kt in range(part * 4, part * 4 + 4):
                        pp = ps_v.tile([128, 512], F32, tag="pjv", bufs=2,
                                       name=f"pjv_{kt}")
                        for eo in range(8):
                            nc.tensor.matmul(
                                pp[:],
                                xv_part[:, eo, (kt % 4) * 128:(kt % 4 + 1) * 128],
                                w_sb["wvT"][:, eo, :],
                                start=(eo == 0), stop=(eo == 7))
                        nc.vector.tensor_tensor(
                            v_all[:, kt, :], pp[:], bv_sb[:],
                            mybir.AluOpType.add)
                    yield

            def gen_A(pair):
                # attn writes are emitted one step late so the SP trigger
                # never waits on the still-running normalize chain
                pending = []
                for qt in range(NQ):
                    for w in pending:
                        w()
                    pending = []
                    for hh in range(2):
                        h = pair * 2 + hh
                        hp = hh * 64
                        expA = work.tile([128, S], BF16, tag="expA", bufs=15,
                                         name=f"expA_{h}_{qt}")
                        zacc = work.tile([128, NHALF], F32, tag="zacc", bufs=6,
                                         name=f"zacc_{h}_{qt}")
                        for half in range(NHALF):
                            scores = ps_sc.tile([128, STRIP], F32, tag="sc",
                                                name=f"sc_{h}_{qt}_{half}")
                            for ki in range(STRIP // 512):
                                k0 = half * STRIP + ki * 512
                                nc.tensor.matmul(
                                    scores[:, ki * 512:(ki + 1) * 512],
                                    qt_all[hp:hp + 64, pair,
                                           qt * 128:(qt + 1) * 128],
                                    kt_all[hp:hp + 64, pair, k0:k0 + 512],
                                    start=True, stop=True)
                            nc.scalar.activation(
                                expA[:, half * STRIP:(half + 1) * STRIP],
                                scores[:], EXPFN, scale=0.125,
                                accum_out=zacc[:, half:half + 1])
                        rz = work.tile([128, 1], F32, tag="rz", bufs=6,
                                       name=f"rz_{h}_{qt}")
                        nc.vector.tensor_tensor(rz[:], zacc[:, 0:1],
                                                zacc[:, 1:2],
                                                mybir.AluOpType.add)
                        nc.vector.reciprocal(rz[:], rz[:])
                        nc.vector.tensor_scalar_mul(expA[:], expA[:], rz[:])
                        pending.append(
                            lambda h=h, qt=qt, expA=expA: nc.sync.dma_start(
                                attn_d[h][qt * 128:(qt + 1) * 128, :], expA[:]))
                    yield
                for w in pending:
                    w()

            PREFETCH = 2

            def gen_ctx(pair):
                psc = ps_ctx.tile([128, S], F32, tag="ctx", name=f"psc_{pair}")
                ats = {}
                for step in range(NT + PREFETCH):
                    if step < NT:
                        for hh in range(2):
                            h = pair * 2 + hh
                            at = work.tile([128, S], BF16, tag="at", bufs=4,
                                           name=f"at_{h}_{step}")
                            nc.sync.dma_start_transpose(
                                at[:],
                                attn_d[h][:, step * 128:(step + 1) * 128])
                            ats[(step, hh)] = at
                    kt = step - PREFETCH
                    if kt >= 0:
                        for hh in range(2):
                            at = ats.pop((kt, hh))
                            for qs in range(NS):
                                nc.tensor.matmul(
                                    psc[hh * 64:(hh + 1) * 64,
                                        qs * 512:(qs + 1) * 512],
                                    v_all[:, kt,
                                          pair * 128 + hh * 64:pair * 128 + hh * 64 + 64],
                                    at[:, qs * 512:(qs + 1) * 512],
                                    start=(kt == 0), stop=(kt == NT - 1),
                                    tile_position=(0, hh * 64),
                                    skip_group_check=True)
                    yield
                for qs in range(NS):
                    nc.vector.tensor_copy(ctxT[:, pair, qs * 512:(qs + 1) * 512],
                                          psc[:, qs * 512:(qs + 1) * 512])
                yield

            def finish_v(vgen):
                nonlocal ps_ctx
                for _ in vgen:
                    pass
                v_stack.close()
                ps_ctx = ctx_stack.enter_context(
                    tc.tile_pool(name="ps_ctx", bufs=1, space="PSUM"))

            vgen = gen_V()
            for pair in range(4):
                for i, _ in enumerate(gen_A(pair)):
                    if pair == 0 and i % 4 == 0:
                        next(vgen, None)
                if pair == 0:
                    finish_v(vgen)
                for _ in gen_ctx(pair):
                    pass
            ctx_stack.close()

        # ---- output projection (partial over this core's 512 channels)
        with tc.tile_pool(name="ps_out", bufs=4, space="PSUM") as ps_out:
            for qt in range(NQ):
                for n in range(2):
                    pso = ps_out.tile([128, 512], F32, tag="out",
                                      name=f"pso_{qt}_{n}")
                    for co in range(4):
                        nc.tensor.matmul(
                            pso[:], ctxT[:, co, qt * 128:(qt + 1) * 128],
                            woT_sb[:, co, n * 512:(n + 1) * 512],
                            start=(co == 0), stop=(co == 3))
                    osb = work.tile([128, 512], F32, tag="osb", bufs=4,
                                    name=f"osb_{qt}_{n}")
                    nc.vector.tensor_copy(osb[:], pso[:])
                    nc.sync.dma_start(
                        outp[qt * 128:(qt + 1) * 128, n * 512:(n + 1) * 512],
                        osb[:])

    return nc


def _pack_core_inputs(query, key_in, value, Wq, bq, Wk, bk, Wv, bv, Wo, bo):
    """Per-core input dicts for cores 0..7 (core = b*2 + g)."""
    def xT_pack(a):  # [S, E] -> [128, 8, S] bf16
        return np.ascontiguousarray(
            a.T.reshape(8, 128, S).transpose(1, 0, 2).astype(NB))

    def wT_pack(w_rows):  # [512, E] -> [128, 8, 512] bf16
        return np.ascontiguousarray(
            w_rows.T.reshape(8, 128, 512).transpose(1, 0, 2).astype(NB))

    xq = [xT_pack(query[b]) for b in range(B)]
    xk = [xT_pack(key_in[b]) for b in range(B)]
    xv = [xT_pack(value[b]) for b in range(B)]
    per_g = []
    for g in range(2):
        cols = slice(g * 512, (g + 1) * 512)
        per_g.append({
            "wqT": wT_pack(Wq[cols, :]),
            "wkT": wT_pack(Wk[cols, :]),
            "wvT": wT_pack(Wv[cols, :]),
            "woT": np.ascontiguousarray(
                Wo[:, cols].T.reshape(4, 128, 1024).transpose(1, 0, 2).astype(NB)),
            "bq2": np.ascontiguousarray(
                bq[cols].reshape(4, 128).T.astype(np.float32)),
            "bk2": np.ascontiguousarray(
                bk[cols].reshape(4, 128).T.astype(np.float32)),
            "bv2": np.ascontiguousarray(
                np.tile(bv[cols].reshape(1, 512), (128, 1)).astype(np.float32)),
        })
    in_maps = []
    for b in range(B):
        for g in range(2):
            m = {"xqT": xq[b], "xkT": xk[b], "xvT": xv[b]}
            m.update(per_g[g])
            in_maps.append(m)
    return in_maps


_CACHED = {}


def kernel(query, key_in, value, Wq, bq, Wk, bk, Wv, bv, Wo, bo):
    query = np.asarray(query, np.float32)
    key_in = np.asarray(key_in, np.float32)
    value = np.asarray(value, np.float32)
    Wq, bq = np.asarray(Wq, np.float32), np.asarray(bq, np.float32)
    Wk, bk = np.asarray(Wk, np.float32), np.asarray(bk, np.float32)
    Wv, bv = np.asarray(Wv, np.float32), np.asarray(bv, np.float32)
    Wo, bo = np.asarray(Wo, np.float32), np.asarray(bo, np.float32)

    if "nc" not in _CACHED:
        nc = build_core(num_devices=8)
        _split_waits(nc)
        _CACHED["nc"] = nc
    nc = _CACHED["nc"]

    in_maps = _pack_core_inputs(query, key_in, value, Wq, bq, Wk, bk, Wv, bv,
                                Wo, bo)
    r = bass_utils.run_bass_kernel_spmd(nc, in_maps, core_ids=list(range(8)))

    out = np.zeros((B, S, E), np.float32)
    attn = np.zeros((B, H, S, S), np.float32)
    for b in range(B):
        for g in range(2):
            res = r.results[b * 2 + g]
            out[b] += res["outp"]
            for h in range(8):
                attn[b, g * 8 + h] = res[f"attn{h}"].astype(np.float32)
        out[b] += bo
    return out, attn
